# revision 11
# baseline (speedup 1.0000x reference)
"""Trainium2 Bass kernel for a cross-attention decoder block.

Shapes (hardcoded): B=2, LQ=LK=2048, D=512, H=8 heads (hd=64), DFF=2048.

    q = x @ Wq; k = enc @ Wk; v = enc @ Wv            (per batch)
    attn = softmax(q k^T / sqrt(hd)); o = attn v
    out1 = LayerNorm(o + x)
    y = LayerNorm(relu(out1 @ W1 + b1) @ W2 + b2 + out1)

Sharding: row-parallel over the 4096 flattened query rows; 8 cores x 512 rows.
Cores 0-3 take batch 0, cores 4-7 batch 1. Each core computes its batch's full
K/V locally (replicated within the 4-core group) -- no collectives.

Engine budget (TimelineSim cost model): the kernel is PSUM-drain bound --
only ACT (0.833 ns/el) and DVE (1.042 ns/el) can read PSUM; Pool/GPSIMD has
no PSUM port.  Assignment:
  - ACT: softmax exp for chunks 2-7 of every head (fp8 out, Exp activation).
  - DVE: exp-bits hack for chunks 0-1 of every head (one 1024-wide two-op
    tensor_scalar: bits = round(s1*qk + s2) as int16 == bf16 e^(S-3)*(1+-2%)),
    all projection emissions (KT/V/qT psum->fp8, paired 1024-wide), attention
    accumulator drains, LN stats/applies (all-SBUF tensor_scalar runs at 2x).
  - Pool: per-head softmax-normalize + residual (SBUF-side stt), memsets.
  - PE: fp8 DoubleRow matmuls (projections, scores, attnV for ACT chunks,
    FFN1), bf16 (attnV for hack chunks, FFN2, transposes).

Numerics (validated on hw, rel ~1.2e-2 vs the 2e-2 gate):
  - host pre-quantizes operands: x^T/enc^T/Wq/Wk/Wv in fp8e4m3 (pow-2 scales),
    W1 fp8, W2/x in bf16; transposes done on the host for free.
  - KT slabs are uniformly scaled (2^3 k^T); the non-pow2 exp-hack slope is
    applied inside the DVE tensor_scalar (mult op), not folded into KT.
  - attn@V: fp8 DoubleRow for ACT chunks, bf16 for hack chunks, accumulated
    into one PSUM tile; a 16.0 "ones" column yields the softmax denominator.
"""

import sys

sys.path.insert(0, "/opt/trn_rl_repo")

from contextlib import ExitStack

import numpy as np
import ml_dtypes

import concourse.bacc as bacc
import concourse.bass as bass
import concourse.mybir as mybir
from concourse import masks, tile
from concourse.bass_utils import run_bass_kernel_spmd

F32 = mybir.dt.float32
BF16 = mybir.dt.bfloat16
F8 = mybir.dt.float8e4
I16 = mybir.dt.int16
F8NP = ml_dtypes.float8_e4m3fn
BF16NP = ml_dtypes.bfloat16

B, LQ, LK, D, H, DFF = 2, 2048, 2048, 512, 8, 2048
HD = D // H  # 64
N_CORES = 8
ROWS = B * LQ // N_CORES  # 512 query rows per core
RT = ROWS // 128  # 4 row tiles
DT = D // 128  # 4 d tiles
LT = LK // 128  # 16 lk tiles
FT = DFF // 128  # 16 dff tiles
NCH = LT // 2  # 8 chunks per head (2 lk tiles each)
import os
NHACK = int(os.environ.get("KNHACK", "2"))  # chunks 0..NHACK-1 per head: DVE exp-bits hack
OLD_HF = bool(int(os.environ.get("KOLDHF", "0")))  # baseline head_final path
EPS = 1e-5
LN2E = float(np.log(2.0))

EOFF = 3.0  # e' = exp(S - EOFF)
# psum scores = 2^9 * S (S = qk/sqrt(hd)); bits = HACK_MULT*psum + HACK_BIAS
HACK_MULT = (128.0 / LN2E) / 512.0
HACK_BIAS = 16256.0 - 7.0 - EOFF * 128.0 / LN2E
KTW = (LT + 1) * 128  # KT slab width incl. the zero k-tile tail

DoubleRow = mybir.MatmulPerfMode.DoubleRow
Alu = mybir.AluOpType


def build_program(apply_g2b2: bool, add_b2: bool) -> bass.Bass:
    nc = bacc.Bacc(None, target_bir_lowering=False, debug=False)

    xt8_d = nc.dram_tensor("xt8", [128, DT * ROWS], F8, kind="ExternalInput")
    xb_d = nc.dram_tensor("xb", [128, RT * D], BF16, kind="ExternalInput")
    enct8_d = nc.dram_tensor("enct8", [128, DT * LK], F8, kind="ExternalInput")
    wq8_d = nc.dram_tensor("wq8", [128, DT * D], F8, kind="ExternalInput")
    wk8_d = nc.dram_tensor("wk8", [128, DT * D], F8, kind="ExternalInput")
    wv8_d = nc.dram_tensor("wv8", [128, DT * D], F8, kind="ExternalInput")
    w1b_d = nc.dram_tensor("w1b", [128, DT * DFF], F8, kind="ExternalInput")
    w2b_d = nc.dram_tensor("w2b", [128, FT * D], BF16, kind="ExternalInput")
    b1c_d = nc.dram_tensor("b1c", [128, FT], F32, kind="ExternalInput")
    g2_d = nc.dram_tensor("g2", [D], F32, kind="ExternalInput")
    be2_d = nc.dram_tensor("be2", [D], F32, kind="ExternalInput")
    b2_d = nc.dram_tensor("b2", [D], F32, kind="ExternalInput")
    y_d = nc.dram_tensor("y", [128, RT * D], F32, kind="ExternalOutput")

    with ExitStack() as ctx:
        tc = ctx.enter_context(tile.TileContext(nc))
        cpool = ctx.enter_context(tc.tile_pool(name="const", bufs=1))
        wpool = ctx.enter_context(tc.tile_pool(name="w8", bufs=4))
        encpool = ctx.enter_context(tc.tile_pool(name="enc8", bufs=1))
        w1pool = ctx.enter_context(tc.tile_pool(name="w1b", bufs=1))
        w2pool = ctx.enter_context(tc.tile_pool(name="w2b", bufs=1))
        xbpool = ctx.enter_context(tc.tile_pool(name="xb", bufs=1))
        qtpool = ctx.enter_context(tc.tile_pool(name="qt8", bufs=1))
        ktpool = ctx.enter_context(tc.tile_pool(name="kt8", bufs=4))
        vpool = ctx.enter_context(tc.tile_pool(name="v8", bufs=1))
        vbpool = ctx.enter_context(tc.tile_pool(name="vb", bufs=1))
        e8pool = ctx.enter_context(tc.tile_pool(name="e8", bufs=2))
        ebpool = ctx.enter_context(tc.tile_pool(name="ebb", bufs=2))
        o1pool = ctx.enter_context(tc.tile_pool(name="o1", bufs=1))
        accspool = ctx.enter_context(tc.tile_pool(name="accs", bufs=2))
        ob1pool = ctx.enter_context(tc.tile_pool(name="out1b", bufs=1))
        o1tpool = ctx.enter_context(tc.tile_pool(name="o1t", bufs=1))
        h1pool = ctx.enter_context(tc.tile_pool(name="h1t", bufs=1))
        ypool = ctx.enter_context(tc.tile_pool(name="y", bufs=4))
        scrpool = ctx.enter_context(tc.tile_pool(name="scr", bufs=2))
        spool = ctx.enter_context(tc.tile_pool(name="stat", bufs=16))
        # PSUM: pA = 2 slots x 2 banks (warmup/sc/ffn1 + transpose psums),
        # pB = 2 x 1 bank (attnV accums -> pff01),
        # pC = 1 x 2 banks (projection pairs -> pff23).
        pA = ctx.enter_context(tc.tile_pool(name="pA", bufs=2, space="PSUM"))
        pB = ctx.enter_context(tc.tile_pool(name="pB", bufs=2, space="PSUM"))
        pC = ctx.enter_context(tc.tile_pool(name="pC", bufs=1, space="PSUM"))

        # ---- PE warmup through the p-state ramp while the first DMAs land ----
        wsrc = cpool.tile([128, 128], BF16)
        nc.gpsimd.memset(wsrc[:], 0.0)
        for i in range(16):
            wp = pA.tile([128, 128], F32, name=f"warm{i}", tag="pA")
            nc.tensor.matmul(wp[:], wsrc[:], wsrc[:], start=True, stop=True)

        # ---- constants ----
        eps_col = cpool.tile([128, 1], F32)
        nc.gpsimd.memset(eps_col[:], EPS)
        moff_col = cpool.tile([128, 1], F32)
        nc.gpsimd.memset(moff_col[:], -EOFF)

        # ---- input loads (first-needed first) ----
        def load(pool_, name, dram, cols, dt_):
            t = pool_.tile([128, cols], dt_, name=name, tag=name)
            nc.sync.dma_start(t[:], dram[:, :])
            return t

        xt8 = load(wpool, "xt8", xt8_d, DT * ROWS, F8)
        wq8 = load(wpool, "wq8", wq8_d, DT * D, F8)
        wk8 = load(wpool, "wk8", wk8_d, DT * D, F8)
        enct8 = encpool.tile([128, DT * LK], F8, name="enct8", tag="enct8")
        encdv = enct8_d[:, :].rearrange("p (n w) -> p n w", w=LK)
        enctv_ = enct8[:].rearrange("p (n w) -> p n w", w=LK)
        for k in range(4):
            nc.sync.dma_start(
                enctv_[:, :, k * 512 : (k + 1) * 512],
                encdv[:, :, k * 512 : (k + 1) * 512],
            )
        wv8 = load(wpool, "wv8", wv8_d, DT * D, F8)
        xb = load(xbpool, "xb", xb_d, RT * D, BF16)
        b1c = load(cpool, "b1c", b1c_d, FT, F32)
        w1b = load(w1pool, "w1b", w1b_d, DT * DFF, F8)
        w2b = load(w2pool, "w2b", w2b_d, FT * D, BF16)

        xt8v = xt8[:].rearrange("p (n w) -> p n w", w=ROWS)
        wq8v = wq8[:].rearrange("p (n w) -> p n w", w=D)
        wk8v = wk8[:].rearrange("p (n w) -> p n w", w=D)
        wv8v = wv8[:].rearrange("p (n w) -> p n w", w=D)
        enct8v = enct8[:].rearrange("p (n w) -> p n w", w=LK)
        xbv = xb[:].rearrange("p (r d) -> p r d", d=D)

        # ---- qT projection: 2 pairs [128,1024], fp8 DR matmuls, DVE drain ----
        qt8 = qtpool.tile([128, DT * ROWS + ROWS], F8, name="qt8", tag="qt8")
        nc.gpsimd.memset(qt8[:, DT * ROWS :], 0.0)
        qt8v = qt8[:].rearrange("p (n w) -> p n w", w=ROWS)

        def drain(eng, dst, psum, scale):
            if eng is nc.scalar:
                nc.scalar.mul(dst, psum, scale)
            else:
                eng.tensor_scalar(dst, psum, scale, None, Alu.mult)

        def emit_qt_pair(s, pool_=None, eng=None):
            pool_ = pool_ or pC
            eng = eng or nc.vector
            pq = pool_.tile([128, 1024], F32, name=f"pq{s}", tag=pool_.name)
            for si in range(2):
                for j in range(0, DT, 2):
                    nc.tensor.matmul(
                        pq[:, si * 512 : (si + 1) * 512],
                        wq8v[:, j : j + 2, (s + si) * 128 : (s + si + 1) * 128],
                        xt8v[:, j : j + 2, :],
                        start=(j == 0),
                        stop=(j == DT - 2),
                        perf_mode=DoubleRow,
                    )
            drain(eng, qt8[:, s * ROWS : (s + 2) * ROWS], pq[:], 2.0**-6)

        # ---- KT slabs (uniform 2^-6 scale; zero k-tile tail) ----
        kt8 = [
            ktpool.tile([128, KTW], F8, name=f"kt8_{s}", tag="kt8") for s in range(DT)
        ]
        for s in range(DT):
            nc.gpsimd.memset(kt8[s][:, LT * 128 :], 0.0)

        def emit_kt_pair(s, g, pool_=None, eng=None):
            """KT slab s, lk columns [1024*g, 1024*(g+1))."""
            pool_ = pool_ or pC
            eng = eng or nc.vector
            pk = pool_.tile([128, 1024], F32, name=f"pk{s}_{g}", tag=pool_.name)
            for ci in range(2):
                cb = 2 * g + ci
                for j in range(0, DT, 2):
                    nc.tensor.matmul(
                        pk[:, ci * 512 : (ci + 1) * 512],
                        wk8v[:, j : j + 2, s * 128 : (s + 1) * 128],
                        enct8v[:, j : j + 2, cb * 512 : (cb + 1) * 512],
                        start=(j == 0),
                        stop=(j == DT - 2),
                        perf_mode=DoubleRow,
                    )
            drain(eng, kt8[s][:, g * 1024 : (g + 1) * 1024], pk[:], 2.0**-6)

        # ---- V layout: chunks 0-1 (hack) -> vb bf16; chunks 2-7 -> v8 fp8;
        # col 64 = 16.0 softmax-denominator column ----
        v8 = vpool.tile([128, H, NCH - NHACK, 2, 68], F8, name="v8", tag="v8")
        v8f = v8[:].rearrange("p a b c d -> p (a b c) d")
        nc.gpsimd.memset(v8f[:, :, 64:65], 16.0)
        nc.gpsimd.memset(v8f[:, :, 65:68], 0.0)
        if NHACK:
            vb = vbpool.tile([128, H, 2 * NHACK, 68], BF16, name="vb", tag="vb")
            vbf = vb[:].rearrange("p a b c -> p (a b) c")
            nc.gpsimd.memset(vbf[:, :, 64:65], 16.0)
            nc.gpsimd.memset(vbf[:, :, 65:68], 0.0)

        def emit_v_pair(t, eng=None):
            """lk tiles t, t+1 (t even) -> one [128,1024] psum, one drain."""
            eng = eng or nc.vector
            pv = pC.tile([128, 1024], F32, name=f"pv{t}", tag="pC")
            for ti in range(2):
                for j in range(0, DT, 2):
                    nc.tensor.matmul(
                        pv[:, ti * 512 : (ti + 1) * 512],
                        enct8v[:, j : j + 2, (t + ti) * 128 : (t + ti + 1) * 128],
                        wv8v[:, j : j + 2, :],
                        start=(j == 0),
                        stop=(j == DT - 2),
                        perf_mode=DoubleRow,
                    )
            pvh = pv[:].rearrange("p (t h d) -> p h t d", t=2, h=H)
            if t >= 2 * NHACK:
                c = t // 2  # both tiles belong to chunk c (fp8 path)
                drain(eng, v8[:, :, c - NHACK, :, 0:64], pvh, 2.0**-5)
            else:
                drain(eng, vb[:, :, t : t + 2, 0:64], pvh, 2.0**-5)

        # ---- attention (software pipelined, projections interleaved) ----
        o1 = o1pool.tile([128, RT * D], F32, name="o1", tag="o1")
        o1v = o1[:].rearrange("p (r d) -> p r d", d=D)
        e8s = [
            e8pool.tile([128, (NCH - NHACK) * 1024], F8, name=f"e8_{i}", tag="e8")
            for i in range(2)
        ]
        ebbs = [
            ebpool.tile([128, max(NHACK, 1) * 1024], BF16, name=f"ebb{i}", tag="ebb")
            for i in range(2)
        ]
        accs = [None] * H

        def is_hack(h, c):
            return c < NHACK

        def emit_attnv(h, c):
            e8 = e8s[h % 2]
            ebb = ebbs[h % 2]
            e8v = e8[:].rearrange("p (t q) -> p t q", q=512)
            acc = accs[h]
            first = c == FIRST_C
            last = c == LAST_C
            for qt_ in range(RT):
                if not is_hack(h, c):
                    cc = c - NHACK
                    nc.tensor.matmul(
                        acc[:, qt_, :],
                        e8v[:, 2 * cc : 2 * cc + 2, qt_ * 128 : (qt_ + 1) * 128],
                        v8[:, h, cc, :, :],
                        start=(first and qt_ == 0),
                        stop=(last and qt_ == RT - 1),
                        perf_mode=DoubleRow,
                    )
                else:
                    for tt in range(2):
                        tloc = 2 * c + tt
                        nc.tensor.matmul(
                            acc[:, qt_, :],
                            ebb[:, tloc * 512 + qt_ * 128 :][:, :128],
                            vb[:, h, tloc, :],
                            start=(first and qt_ == 0 and tt == 0),
                            stop=(last and qt_ == RT - 1 and tt == 1),
                        )

        def emit_head_final(h):
            if OLD_HF:
                acc = accs[h]
                rec = spool.tile([128, RT], F32, name=f"rec{h}", tag="stat")
                nc.vector.reciprocal(rec[:], acc[:, :, 64:65])
                for qt_ in range(RT):
                    nc.vector.scalar_tensor_tensor(
                        o1v[:, qt_, h * 64 : (h + 1) * 64],
                        acc[:, qt_, 0:64],
                        rec[:, qt_ : qt_ + 1],
                        xbv[:, qt_, h * 64 : (h + 1) * 64],
                        Alu.mult,
                        Alu.add,
                    )
                return
            # wholesale acc drain (DVE) -> reciprocal (DVE) -> Pool scale;
            # the +x residual is added per-row-tile later (DVE 2x, all-SBUF)
            acc = accs[h]
            accS = accspool.tile([128, RT, 68], F32, name=f"accS{h}", tag="accs")
            nc.vector.tensor_copy(accS[:], acc[:])
            rec = spool.tile([128, RT], F32, name=f"rec{h}", tag="stat")
            nc.vector.reciprocal(rec[:], accS[:, :, 64:65])
            for qt_ in range(RT):
                nc.gpsimd.tensor_scalar(
                    o1v[:, qt_, h * 64 : (h + 1) * 64],
                    accS[:, qt_, 0:64],
                    rec[:, qt_ : qt_ + 1],
                    None,
                    Alu.mult,
                )

        # up-front: what gates head 0 (qt s=0,1; KT slab 0), spread across
        # pA slots (free before the score stream) + pC so drains parallelize
        emit_qt_pair(0, pA)
        emit_kt_pair(0, 0, pA, eng=nc.scalar)
        emit_kt_pair(0, 1, pC)

        # chunk emission order per head: ACT chunks lead, hack (DVE) chunks
        # interleave so both drain engines stay fed from a 2-slot sc buffer
        ORDER = [2, 3, 0, 4, 5, 1, 6, 7] if NHACK == 2 else list(range(NCH))
        FIRST_C, LAST_C = ORDER[0], ORDER[-1]
        # just-in-time pair emissions: (head, slot) -> emitter thunk
        JIT = {
            (0, 0): lambda: emit_v_pair(4),
            (0, 1): lambda: emit_v_pair(6, eng=nc.scalar),
            (0, 3): lambda: emit_v_pair(0),
            (0, 5): lambda: emit_v_pair(2, eng=nc.scalar),
            (0, 6): lambda: emit_v_pair(8),
            (1, 0): lambda: emit_v_pair(10, eng=nc.scalar),
            (1, 2): lambda: emit_v_pair(12),
            (1, 4): lambda: emit_v_pair(14, eng=nc.scalar),
            (1, 5): lambda: emit_kt_pair(1, 0),
            (1, 7): lambda: emit_kt_pair(1, 1),
            (2, 1): lambda: emit_qt_pair(2),
            (3, 0): lambda: emit_kt_pair(2, 0),
            (3, 3): lambda: emit_kt_pair(2, 1),
            (5, 0): lambda: emit_kt_pair(3, 0),
            (5, 3): lambda: emit_kt_pair(3, 1),
        }

        pending = []
        for h in range(H):
            pr, off = h // 2, 64 * (h % 2)
            e8 = e8s[h % 2]
            ebb = ebbs[h % 2]
            ktv = kt8[pr][:].rearrange("p (n w) -> p n w", w=128)
            accs[h] = pB.tile([128, RT, 68], F32, name=f"acc{h}", tag="pB")
            for slot, c in enumerate(ORDER):
                jit = JIT.pop((h, slot), None)
                if jit is not None:
                    jit()
                sc = pA.tile([128, 1024], F32, name=f"sc{h}_{c}", tag="pA")
                for tt in range(2):
                    t = 2 * c + tt
                    nc.tensor.matmul(
                        sc[:, tt * 512 : (tt + 1) * 512],
                        ktv[off : off + 64, t : LT + 1 : LT - t, :],
                        qt8v[off : off + 64, pr : DT + 1 : DT - pr, :],
                        start=True,
                        stop=True,
                        perf_mode=DoubleRow,
                        tile_position=(off, 0),
                    )
                if not is_hack(h, c):
                    nc.scalar.activation(
                        e8[:, (c - NHACK) * 1024 : (c - NHACK + 1) * 1024],
                        sc[:],
                        mybir.ActivationFunctionType.Exp,
                        bias=moff_col[:, 0:1],
                        scale=2.0**-9,
                    )
                else:
                    nc.vector.tensor_scalar(
                        ebb[:, c * 1024 : (c + 1) * 1024].bitcast(I16),
                        sc[:],
                        HACK_MULT,
                        HACK_BIAS,
                        Alu.mult,
                        Alu.add,
                    )
                pending.append((h, c))
                if len(pending) > 7:
                    ph_, pc_ = pending.pop(0)
                    emit_attnv(ph_, pc_)
                    if pc_ == NCH - 1:
                        emit_head_final(ph_)
        for ph_, pc_ in pending:
            emit_attnv(ph_, pc_)
            if pc_ == NCH - 1:
                emit_head_final(ph_)

        # ---- LN1 -> out1 (bf16); wave-emitted so row-tiles pipeline ----
        out1b = ob1pool.tile([128, RT * D], BF16, name="out1b", tag="out1b")
        ob1v = out1b[:].rearrange("p (r d) -> p r d", d=D)

        def layer_norm(dst, src, name, gain_bc=None, bias_bc=None):
            bn6 = spool.tile([128, 6], F32, name=f"bn6{name}", tag="stat")
            nc.vector.bn_stats(bn6[:], src)
            mv = spool.tile([128, 2], F32, name=f"mv{name}", tag="stat")
            nc.vector.bn_aggr(mv[:], bn6[:])
            std = spool.tile([128, 1], F32, name=f"std{name}", tag="stat")
            nc.scalar.activation(
                std[:], mv[:, 1:2], mybir.ActivationFunctionType.Sqrt,
                bias=eps_col[:, 0:1],
            )
            rstd = spool.tile([128, 1], F32, name=f"rstd{name}", tag="stat")
            nc.vector.reciprocal(rstd[:], std[:])
            nc.gpsimd.tensor_scalar(
                dst, src, mv[:, 0:1], rstd[:, 0:1], Alu.subtract, Alu.mult
            )
            if gain_bc is not None:
                nc.gpsimd.tensor_tensor(dst, dst, gain_bc[:], Alu.mult)
                nc.gpsimd.tensor_tensor(dst, dst, bias_bc[:], Alu.add)

        bn6s, mvs, stds, rstds = [], [], [], []
        if not OLD_HF:
            for qt_ in range(RT):
                nc.vector.tensor_tensor(
                    o1v[:, qt_, :], o1v[:, qt_, :], xbv[:, qt_, :], Alu.add
                )
        for qt_ in range(RT):
            bn6 = spool.tile([128, 6], F32, name=f"bn6l1_{qt_}", tag="stat")
            nc.vector.bn_stats(bn6[:], o1v[:, qt_, :])
            bn6s.append(bn6)
        for qt_ in range(RT):
            mv = spool.tile([128, 2], F32, name=f"mvl1_{qt_}", tag="stat")
            nc.vector.bn_aggr(mv[:], bn6s[qt_][:])
            mvs.append(mv)
        for qt_ in range(RT):
            std = spool.tile([128, 1], F32, name=f"stdl1_{qt_}", tag="stat")
            nc.scalar.activation(
                std[:], mvs[qt_][:, 1:2], mybir.ActivationFunctionType.Sqrt,
                bias=eps_col[:, 0:1],
            )
            stds.append(std)
        for qt_ in range(RT):
            rstd = spool.tile([128, 1], F32, name=f"rstdl1_{qt_}", tag="stat")
            nc.vector.reciprocal(rstd[:], stds[qt_][:])
            rstds.append(rstd)
        for qt_ in range(RT):
            nc.vector.tensor_scalar(
                ob1v[:, qt_, :], o1v[:, qt_, :], mvs[qt_][:, 0:1],
                rstds[qt_][:, 0:1], Alu.subtract, Alu.mult,
            )

        # ---- out1^T (PE transpose via bf16 psum); rt-outer so each row
        # tile transposes right after its LN1 apply ----
        ident = cpool.tile([128, 128], F32)
        masks.make_identity(nc, ident[:])
        identb = cpool.tile([128, 128], BF16)
        nc.vector.tensor_copy(identb[:], ident[:])
        o1t = o1tpool.tile([128, DT * ROWS], F8, name="o1t", tag="o1t")
        o1tv = o1t[:].rearrange("p (n w) -> p n w", w=ROWS)
        pts01 = pB.tile([128, 2, ROWS], BF16, name="po1t01", tag="pB")
        pts23 = pC.tile([128, 2, ROWS], BF16, name="po1t23", tag="pC")

        def pt_ap(dt_):
            return pts01[:, dt_, :] if dt_ < 2 else pts23[:, dt_ - 2, :]

        for rt_ in range(RT):
            for dt_ in range(DT):
                nc.tensor.matmul(
                    pt_ap(dt_)[:, rt_ * 128 : (rt_ + 1) * 128],
                    ob1v[:, rt_, dt_ * 128 : (dt_ + 1) * 128],
                    identb[:],
                    is_transpose=True,
                    start=(rt_ == 0),
                    stop=(rt_ == RT - 1),
                )
        for dt_ in range(DT):
            # out1 (bf16 psum) -> fp8 x2^2 for the DoubleRow FFN1
            if dt_ < 2:
                nc.scalar.mul(o1tv[:, dt_, :], pt_ap(dt_), 4.0)
            else:
                nc.vector.tensor_scalar(o1tv[:, dt_, :], pt_ap(dt_), 4.0, None, Alu.mult)

        # ---- FFN (fp8 DR FFN1, bf16 FFN2), two row-half passes ----
        h1t = h1pool.tile([128, FT * ROWS], BF16, name="h1t", tag="h1t")
        h1v = h1t[:].rearrange("p (n w) -> p n w", w=ROWS)
        w1v = w1b[:].rearrange("p (n w) -> p n w", w=DFF)
        w2v = w2b[:].rearrange("p (n w) -> p n w", w=D)
        pff0 = pB.tile([128, D], F32, name="pff0", tag="pB")
        pff1 = pB.tile([128, D], F32, name="pff1", tag="pB")
        pff23 = pC.tile([128, 2, D], F32, name="pff23", tag="pC")

        def pff_ap(rt_):
            return (pff0[:], pff1[:], pff23[:, 0, :], pff23[:, 1, :])[rt_]

        g2bc = be2bc = b2bc = None
        if apply_g2b2 or add_b2:
            def bcast(name, dram):
                row = cpool.tile([1, D], F32, name=f"{name}row")
                nc.sync.dma_start(row[:], dram[None, :])
                full = cpool.tile([128, D], F32, name=f"{name}bc")
                nc.gpsimd.partition_broadcast(full[:], row[:])
                return full

            g2bc = bcast("g2", g2_d)
            be2bc = bcast("be2", be2_d)
            b2bc = bcast("b2", b2_d)

        scr = [
            scrpool.tile([128, D], F32, name=f"scr{i}", tag="scr") for i in range(2)
        ]

        def emit_tail(rts):
            if add_b2:
                for rt_ in rts:
                    yt = ypool.tile([128, D], F32, name=f"y{rt_}", tag="y")
                    nc.vector.tensor_tensor(
                        yt[:], pff_ap(rt_), ob1v[:, rt_, :], Alu.add
                    )
                    nc.vector.tensor_tensor(yt[:], yt[:], b2bc[:], Alu.add)
                    layer_norm(
                        yt[:], yt[:], f"ln2_{rt_}",
                        gain_bc=g2bc if apply_g2b2 else None,
                        bias_bc=be2bc if apply_g2b2 else None,
                    )
                    nc.sync.dma_start(y_d[:, rt_ * D : (rt_ + 1) * D], yt[:])
                return
            yts, s1s, s2s, uss, mus = {}, {}, {}, {}, {}
            for rt_ in rts:
                yt = ypool.tile([128, D], F32, name=f"y{rt_}", tag="y")
                s1 = spool.tile([128, 1], F32, name=f"s1_{rt_}", tag="stat")
                nc.vector.scalar_tensor_tensor(
                    yt[:], pff_ap(rt_), 1.0, ob1v[:, rt_, :], Alu.mult, Alu.add,
                    accum_out=s1[:],
                )
                yts[rt_], s1s[rt_] = yt, s1
            for rt_ in rts:
                s2 = spool.tile([128, 1], F32, name=f"s2_{rt_}", tag="stat")
                nc.scalar.activation(
                    scr[rt_ % 2][:], yts[rt_][:],
                    mybir.ActivationFunctionType.Square, accum_out=s2[:],
                )
                s2s[rt_] = s2
            for rt_ in rts:
                # var = (s2 - s1^2/D)/D; std = sqrt(var + eps)
                u = spool.tile([128, 1], F32, name=f"u{rt_}", tag="stat")
                nc.vector.tensor_tensor(u[:], s1s[rt_][:], s1s[rt_][:], Alu.mult)
                nc.vector.tensor_scalar(u[:], u[:], 1.0 / D, None, Alu.mult)
                nc.vector.tensor_tensor(u[:], s2s[rt_][:], u[:], Alu.subtract)
                uss[rt_] = u
                mu = spool.tile([128, 1], F32, name=f"mu{rt_}", tag="stat")
                nc.vector.tensor_scalar(mu[:], s1s[rt_][:], 1.0 / D, None, Alu.mult)
                mus[rt_] = mu
            stds2 = {}
            for rt_ in rts:
                std = spool.tile([128, 1], F32, name=f"stdy{rt_}", tag="stat")
                nc.scalar.activation(
                    std[:], uss[rt_][:], mybir.ActivationFunctionType.Sqrt,
                    bias=eps_col[:, 0:1], scale=1.0 / D,
                )
                stds2[rt_] = std
            for rt_ in rts:
                rstd = spool.tile([128, 1], F32, name=f"rstdy{rt_}", tag="stat")
                nc.vector.reciprocal(rstd[:], stds2[rt_][:])
                nc.vector.tensor_scalar(
                    yts[rt_][:], yts[rt_][:], mus[rt_][:, 0:1],
                    rstd[:, 0:1], Alu.subtract, Alu.mult,
                )
                if apply_g2b2:
                    nc.vector.tensor_tensor(yts[rt_][:], yts[rt_][:], g2bc[:], Alu.mult)
                    nc.vector.tensor_tensor(yts[rt_][:], yts[rt_][:], be2bc[:], Alu.add)
                nc.sync.dma_start(y_d[:, rt_ * D : (rt_ + 1) * D], yts[rt_][:])

        HALF = ROWS // 2  # 256 rows per pass
        for half in range(2):
            r0 = half * HALF
            rts = [2 * half, 2 * half + 1]

            def emit_ffn2(s):
                for rt_ in rts:
                    nc.tensor.matmul(
                        pff_ap(rt_),
                        h1v[:, s, rt_ * 128 : (rt_ + 1) * 128],
                        w2v[:, s, :],
                        start=(s == 0),
                        stop=(s == FT - 1),
                    )

            for s in range(FT):
                ph = pA.tile([128, HALF], F32, name=f"ph{half}_{s}", tag="pA")
                for j in range(0, DT, 2):
                    nc.tensor.matmul(
                        ph[:],
                        w1v[:, j : j + 2, s * 128 : (s + 1) * 128],
                        o1tv[:, j : j + 2, r0 : r0 + HALF],
                        start=(j == 0),
                        stop=(j == DT - 2),
                        perf_mode=DoubleRow,
                    )
                # psum = 2^9 h1pre; h1 = relu(h1pre + b1) in bf16
                nc.scalar.activation(
                    h1v[:, s, r0 : r0 + HALF], ph[:],
                    mybir.ActivationFunctionType.Relu, bias=b1c[:, s : s + 1],
                    scale=2.0**-9,
                )
                if s > 1:
                    emit_ffn2(s - 2)
            emit_ffn2(FT - 2)
            emit_ffn2(FT - 1)
            emit_tail(rts)

    nc.compile()
    return nc


_CACHED = {}


def _get_nc(apply_g2b2: bool = False, add_b2: bool = False):
    key = (apply_g2b2, add_b2)
    if key not in _CACHED:
        _CACHED[key] = build_program(*key)
    return _CACHED[key]


def _f8(x, scale_pow):
    return (np.asarray(x, np.float32) * (2.0**scale_pow)).astype(F8NP)


def _ktile_rows(a):
    """[K, M] -> [128, (K//128)*M]: out[p, j*M + m] = a[j*128 + p, m]."""
    K, M = a.shape
    return np.ascontiguousarray(
        a.reshape(K // 128, 128, M).transpose(1, 0, 2).reshape(128, -1)
    )


def kernel(**inputs) -> np.ndarray:
    x = np.asarray(inputs["inputs"], dtype=np.float32)
    enc = np.asarray(inputs["encoder_x"], dtype=np.float32)
    assert x.shape == (B, LQ, D) and enc.shape == (B, LK, D)
    assert int(np.asarray(inputs["n_heads"])) == H

    Wq = np.asarray(inputs["Wq"], np.float32)
    Wk = np.asarray(inputs["Wk"], np.float32)
    Wv = np.asarray(inputs["Wv"], np.float32)
    g1 = np.asarray(inputs["ln1_g"], np.float64)
    be1 = np.asarray(inputs["ln1_b"], np.float64)
    w1_raw = np.asarray(inputs["W1"], np.float64)
    w1_eff = (g1[:, None] * w1_raw).astype(np.float32)
    b1_eff = (np.asarray(inputs["b1"], np.float64) + be1 @ w1_raw).astype(np.float32)
    W2 = np.asarray(inputs["W2"], np.float32)
    b2 = np.asarray(inputs["b2"], np.float32)
    g2 = np.asarray(inputs["ln2_g"], np.float32)
    be2 = np.asarray(inputs["ln2_b"], np.float32)

    apply_g2b2 = not (np.allclose(g2, 1.0) and np.allclose(be2, 0.0))
    add_b2 = not np.allclose(b2, 0.0)
    nc = _get_nc(apply_g2b2, add_b2)

    shared = {
        "wq8": _ktile_rows(_f8(Wq, 5)),
        "wk8": _ktile_rows(_f8(Wk, 5)),
        "wv8": _ktile_rows(_f8(Wv, 5)),
        "w1b": _ktile_rows(_f8(w1_eff, 7)),
        "w2b": _ktile_rows(W2.astype(BF16NP)),
        "b1c": np.ascontiguousarray(_ktile_rows(b1_eff[:, None]).astype(np.float32)),
        "g2": np.ascontiguousarray(g2),
        "be2": np.ascontiguousarray(be2),
        "b2": np.ascontiguousarray(b2),
    }
    xf = x.reshape(B * LQ, D)
    in_maps = []
    for c in range(N_CORES):
        b = c // (N_CORES // B)
        xs = xf[c * ROWS : (c + 1) * ROWS]
        m = dict(shared)
        m["xt8"] = _ktile_rows(_f8(np.ascontiguousarray(xs.T), 4))
        m["xb"] = _ktile_rows(xs.astype(BF16NP))
        m["enct8"] = _ktile_rows(_f8(np.ascontiguousarray(enc[b].T), 4))
        in_maps.append(m)

    res = run_bass_kernel_spmd(nc, in_maps, core_ids=list(range(N_CORES)))
    out = np.empty((B * LQ, D), np.float32)
    for c in range(N_CORES):
        yc = res.results[c]["y"].reshape(128, RT, D).transpose(1, 0, 2).reshape(ROWS, D)
        out[c * ROWS : (c + 1) * ROWS] = yc
    return out.reshape(B, LQ, D)


# revision 13
# speedup vs baseline: 1.0257x; 1.0257x over previous
"""Trainium2 Bass kernel for a cross-attention decoder block.

Shapes (hardcoded): B=2, LQ=LK=2048, D=512, H=8 heads (hd=64), DFF=2048.

    q = x @ Wq; k = enc @ Wk; v = enc @ Wv            (per batch)
    attn = softmax(q k^T / sqrt(hd)); o = attn v
    out1 = LayerNorm(o + x)
    y = LayerNorm(relu(out1 @ W1 + b1) @ W2 + b2 + out1)

Sharding: row-parallel over the 4096 flattened query rows; 8 cores x 512 rows.
Cores 0-3 take batch 0, cores 4-7 batch 1. Each core computes its batch's full
K/V locally (replicated within the 4-core group) -- no collectives.

Engine budget (TimelineSim cost model): the kernel is PSUM-drain bound --
only ACT (0.833 ns/el) and DVE (1.042 ns/el) can read PSUM; Pool/GPSIMD has
no PSUM port.  Assignment:
  - ACT: softmax exp for chunks 2-7 of every head (fp8 out, Exp activation).
  - DVE: exp-bits hack for chunks 0-1 of every head (one 1024-wide two-op
    tensor_scalar: bits = round(s1*qk + s2) as int16 == bf16 e^(S-3)*(1+-2%)),
    all projection emissions (KT/V/qT psum->fp8, paired 1024-wide), attention
    accumulator drains, LN stats/applies (all-SBUF tensor_scalar runs at 2x).
  - Pool: per-head softmax-normalize + residual (SBUF-side stt), memsets.
  - PE: fp8 DoubleRow matmuls (projections, scores, attnV for ACT chunks,
    FFN1), bf16 (attnV for hack chunks, FFN2, transposes).

Numerics (validated on hw, rel ~1.2e-2 vs the 2e-2 gate):
  - host pre-quantizes operands: x^T/enc^T/Wq/Wk/Wv in fp8e4m3 (pow-2 scales),
    W1 fp8, W2/x in bf16; transposes done on the host for free.
  - KT slabs are uniformly scaled (2^3 k^T); the non-pow2 exp-hack slope is
    applied inside the DVE tensor_scalar (mult op), not folded into KT.
  - attn@V: fp8 DoubleRow for ACT chunks, bf16 for hack chunks, accumulated
    into one PSUM tile; a 16.0 "ones" column yields the softmax denominator.
"""

import sys

sys.path.insert(0, "/opt/trn_rl_repo")

from contextlib import ExitStack

import numpy as np
import ml_dtypes

import concourse.bacc as bacc
import concourse.bass as bass
import concourse.mybir as mybir
from concourse import masks, tile
from concourse.bass_utils import run_bass_kernel_spmd

F32 = mybir.dt.float32
BF16 = mybir.dt.bfloat16
F8 = mybir.dt.float8e4
I16 = mybir.dt.int16
F8NP = ml_dtypes.float8_e4m3fn
BF16NP = ml_dtypes.bfloat16

B, LQ, LK, D, H, DFF = 2, 2048, 2048, 512, 8, 2048
HD = D // H  # 64
N_CORES = 8
ROWS = B * LQ // N_CORES  # 512 query rows per core
RT = ROWS // 128  # 4 row tiles
DT = D // 128  # 4 d tiles
LT = LK // 128  # 16 lk tiles
FT = DFF // 128  # 16 dff tiles
NCH = LT // 2  # 8 chunks per head (2 lk tiles each)
import os
NHACK = int(os.environ.get("KNHACK", "2"))  # chunks 0..NHACK-1 per head: DVE exp-bits hack
OLD_HF = bool(int(os.environ.get("KOLDHF", "0")))  # baseline head_final path
EPS = 1e-5
LN2E = float(np.log(2.0))

EOFF = 3.0  # e' = exp(S - EOFF)
# psum scores = 2^9 * S (S = qk/sqrt(hd)); bits = HACK_MULT*psum + HACK_BIAS
HACK_MULT = (128.0 / LN2E) / 512.0
HACK_BIAS = 16256.0 - 7.0 - EOFF * 128.0 / LN2E
KTW = (LT + 1) * 128  # KT slab width incl. the zero k-tile tail

DoubleRow = mybir.MatmulPerfMode.DoubleRow
Alu = mybir.AluOpType


def build_program(apply_g2b2: bool, add_b2: bool) -> bass.Bass:
    nc = bacc.Bacc(None, target_bir_lowering=False, debug=False)

    xt8_d = nc.dram_tensor("xt8", [128, DT * ROWS], F8, kind="ExternalInput")
    xb_d = nc.dram_tensor("xb", [128, RT * D], BF16, kind="ExternalInput")
    enct8_d = nc.dram_tensor("enct8", [128, DT * LK], F8, kind="ExternalInput")
    wq8_d = nc.dram_tensor("wq8", [128, DT * D], F8, kind="ExternalInput")
    wk8_d = nc.dram_tensor("wk8", [128, DT * D], F8, kind="ExternalInput")
    wv8_d = nc.dram_tensor("wv8", [128, DT * D], F8, kind="ExternalInput")
    w1b_d = nc.dram_tensor("w1b", [128, DT * DFF], F8, kind="ExternalInput")
    w2b_d = nc.dram_tensor("w2b", [128, FT * D], BF16, kind="ExternalInput")
    b1c_d = nc.dram_tensor("b1c", [128, FT], F32, kind="ExternalInput")
    g2_d = nc.dram_tensor("g2", [D], F32, kind="ExternalInput")
    be2_d = nc.dram_tensor("be2", [D], F32, kind="ExternalInput")
    b2_d = nc.dram_tensor("b2", [D], F32, kind="ExternalInput")
    y_d = nc.dram_tensor("y", [128, RT * D], F32, kind="ExternalOutput")

    with ExitStack() as ctx:
        tc = ctx.enter_context(tile.TileContext(nc))
        cpool = ctx.enter_context(tc.tile_pool(name="const", bufs=1))
        wpool = ctx.enter_context(tc.tile_pool(name="w8", bufs=4))
        encpool = ctx.enter_context(tc.tile_pool(name="enc8", bufs=1))
        w1pool = ctx.enter_context(tc.tile_pool(name="w1b", bufs=1))
        w2pool = ctx.enter_context(tc.tile_pool(name="w2b", bufs=1))
        xbpool = ctx.enter_context(tc.tile_pool(name="xb", bufs=1))
        qtpool = ctx.enter_context(tc.tile_pool(name="qt8", bufs=1))
        ktpool = ctx.enter_context(tc.tile_pool(name="kt8", bufs=4))
        vpool = ctx.enter_context(tc.tile_pool(name="v8", bufs=1))
        vbpool = ctx.enter_context(tc.tile_pool(name="vb", bufs=1))
        e8pool = ctx.enter_context(tc.tile_pool(name="e8", bufs=2))
        ebpool = ctx.enter_context(tc.tile_pool(name="ebb", bufs=2))
        o1pool = ctx.enter_context(tc.tile_pool(name="o1", bufs=1))
        accspool = ctx.enter_context(tc.tile_pool(name="accs", bufs=2))
        ob1pool = ctx.enter_context(tc.tile_pool(name="out1b", bufs=1))
        o1tpool = ctx.enter_context(tc.tile_pool(name="o1t", bufs=1))
        h1pool = ctx.enter_context(tc.tile_pool(name="h1t", bufs=1))
        ypool = ctx.enter_context(tc.tile_pool(name="y", bufs=4))
        scrpool = ctx.enter_context(tc.tile_pool(name="scr", bufs=2))
        spool = ctx.enter_context(tc.tile_pool(name="stat", bufs=16))
        # PSUM: pA = 2 slots x 2 banks (sc chunks; later ffn1/transpose),
        # pB = 2 x 1 bank (attnV accums -> pff01/pts01),
        # pC = 2 x 1 bank (projection 512-wide units -> pff23/pts23).
        pA = ctx.enter_context(tc.tile_pool(name="pA", bufs=2, space="PSUM"))
        pB = ctx.enter_context(tc.tile_pool(name="pB", bufs=2, space="PSUM"))
        pC = ctx.enter_context(tc.tile_pool(name="pC", bufs=2, space="PSUM"))

        # ---- PE warmup through the p-state ramp while the first DMAs land ----
        wsrc = cpool.tile([128, 128], BF16)
        nc.gpsimd.memset(wsrc[:], 0.0)
        for i in range(16):
            wp = pA.tile([128, 128], F32, name=f"warm{i}", tag="pA")
            nc.tensor.matmul(wp[:], wsrc[:], wsrc[:], start=True, stop=True)

        # ---- constants ----
        eps_col = cpool.tile([128, 1], F32)
        nc.gpsimd.memset(eps_col[:], EPS)
        moff_col = cpool.tile([128, 1], F32)
        nc.gpsimd.memset(moff_col[:], -EOFF)

        # ---- input loads (first-needed first) ----
        def load(pool_, name, dram, cols, dt_):
            t = pool_.tile([128, cols], dt_, name=name, tag=name)
            nc.sync.dma_start(t[:], dram[:, :])
            return t

        xt8 = load(wpool, "xt8", xt8_d, DT * ROWS, F8)
        wq8 = load(wpool, "wq8", wq8_d, DT * D, F8)
        wk8 = load(wpool, "wk8", wk8_d, DT * D, F8)
        wv8 = load(wpool, "wv8", wv8_d, DT * D, F8)
        enct8 = encpool.tile([128, DT * LK], F8, name="enct8", tag="enct8")
        encdv = enct8_d[:, :].rearrange("p (n w) -> p n w", w=LK)
        enctv_ = enct8[:].rearrange("p (n w) -> p n w", w=LK)
        for k in (1, 0, 2, 3):
            nc.sync.dma_start(
                enctv_[:, :, k * 512 : (k + 1) * 512],
                encdv[:, :, k * 512 : (k + 1) * 512],
            )
        xb = load(xbpool, "xb", xb_d, RT * D, BF16)
        b1c = load(cpool, "b1c", b1c_d, FT, F32)
        w1b = load(w1pool, "w1b", w1b_d, DT * DFF, F8)
        w2b = load(w2pool, "w2b", w2b_d, FT * D, BF16)

        xt8v = xt8[:].rearrange("p (n w) -> p n w", w=ROWS)
        wq8v = wq8[:].rearrange("p (n w) -> p n w", w=D)
        wk8v = wk8[:].rearrange("p (n w) -> p n w", w=D)
        wv8v = wv8[:].rearrange("p (n w) -> p n w", w=D)
        enct8v = enct8[:].rearrange("p (n w) -> p n w", w=LK)
        xbv = xb[:].rearrange("p (r d) -> p r d", d=D)

        # ---- qT projection: 2 pairs [128,1024], fp8 DR matmuls, DVE drain ----
        qt8 = qtpool.tile([128, DT * ROWS + ROWS], F8, name="qt8", tag="qt8")
        nc.gpsimd.memset(qt8[:, DT * ROWS :], 0.0)
        qt8v = qt8[:].rearrange("p (n w) -> p n w", w=ROWS)

        def drain(eng, dst, psum, scale):
            if eng is nc.scalar:
                nc.scalar.mul(dst, psum, scale)
            else:
                eng.tensor_scalar(dst, psum, scale, None, Alu.mult)

        def emit_qt_unit(s, eng=None):
            pq = pC.tile([128, 512], F32, name=f"pq{s}", tag="pC")
            for j in range(0, DT, 2):
                nc.tensor.matmul(
                    pq[:],
                    wq8v[:, j : j + 2, s * 128 : (s + 1) * 128],
                    xt8v[:, j : j + 2, :],
                    start=(j == 0),
                    stop=(j == DT - 2),
                    perf_mode=DoubleRow,
                )
            drain(eng or nc.vector, qt8[:, s * ROWS : (s + 1) * ROWS], pq[:], 2.0**-6)

        # ---- KT slabs (uniform 2^-6 scale; zero k-tile tail) ----
        kt8 = [
            ktpool.tile([128, KTW], F8, name=f"kt8_{s}", tag="kt8") for s in range(DT)
        ]
        for s in range(DT):
            nc.gpsimd.memset(kt8[s][:, LT * 128 :], 0.0)

        def emit_kt_unit(s, cb, eng=None):
            """KT slab s, lk columns [512*cb, 512*(cb+1))."""
            pk = pC.tile([128, 512], F32, name=f"pk{s}_{cb}", tag="pC")
            for j in range(0, DT, 2):
                nc.tensor.matmul(
                    pk[:],
                    wk8v[:, j : j + 2, s * 128 : (s + 1) * 128],
                    enct8v[:, j : j + 2, cb * 512 : (cb + 1) * 512],
                    start=(j == 0),
                    stop=(j == DT - 2),
                    perf_mode=DoubleRow,
                )
            drain(eng or nc.vector, kt8[s][:, cb * 512 : (cb + 1) * 512], pk[:], 2.0**-6)

        # ---- V layout: chunks 0-1 (hack) -> vb bf16; chunks 2-7 -> v8 fp8;
        # col 64 = 16.0 softmax-denominator column ----
        v8 = vpool.tile([128, H, NCH - NHACK, 2, 68], F8, name="v8", tag="v8")
        v8f = v8[:].rearrange("p a b c d -> p (a b c) d")
        nc.gpsimd.memset(v8f[:, :, 64:65], 16.0)
        nc.gpsimd.memset(v8f[:, :, 65:68], 0.0)
        if NHACK:
            vb = vbpool.tile([128, H, 2 * NHACK, 68], BF16, name="vb", tag="vb")
            vbf = vb[:].rearrange("p a b c -> p (a b) c")
            nc.gpsimd.memset(vbf[:, :, 64:65], 16.0)
            nc.gpsimd.memset(vbf[:, :, 65:68], 0.0)

        def emit_v_unit(t, eng=None):
            """lk tile t -> one [128,512] psum, one drain."""
            pv = pC.tile([128, 512], F32, name=f"pv{t}", tag="pC")
            for j in range(0, DT, 2):
                nc.tensor.matmul(
                    pv[:],
                    enct8v[:, j : j + 2, t * 128 : (t + 1) * 128],
                    wv8v[:, j : j + 2, :],
                    start=(j == 0),
                    stop=(j == DT - 2),
                    perf_mode=DoubleRow,
                )
            pvh = pv[:].rearrange("p (h d) -> p h d", h=H)
            if t >= 2 * NHACK:
                drain(eng or nc.vector, v8[:, :, t // 2 - NHACK, t % 2, 0:64], pvh, 2.0**-5)
            else:
                drain(eng or nc.vector, vb[:, :, t, 0:64], pvh, 2.0**-5)

        # ---- attention (software pipelined, projections interleaved) ----
        o1 = o1pool.tile([128, RT * D], F32, name="o1", tag="o1")
        o1v = o1[:].rearrange("p (r d) -> p r d", d=D)
        e8s = [
            e8pool.tile([128, (NCH - NHACK) * 1024], F8, name=f"e8_{i}", tag="e8")
            for i in range(2)
        ]
        ebbs = [
            ebpool.tile([128, max(NHACK, 1) * 1024], BF16, name=f"ebb{i}", tag="ebb")
            for i in range(2)
        ]
        accs = [None] * H

        def is_hack(h, c):
            return c < NHACK

        def emit_attnv(h, c):
            e8 = e8s[h % 2]
            ebb = ebbs[h % 2]
            e8v = e8[:].rearrange("p (t q) -> p t q", q=512)
            acc = accs[h]
            first = c == FIRST_C
            last = c == LAST_C
            for qt_ in range(RT):
                if not is_hack(h, c):
                    cc = c - NHACK
                    nc.tensor.matmul(
                        acc[:, qt_, :],
                        e8v[:, 2 * cc : 2 * cc + 2, qt_ * 128 : (qt_ + 1) * 128],
                        v8[:, h, cc, :, :],
                        start=(first and qt_ == 0),
                        stop=(last and qt_ == RT - 1),
                        perf_mode=DoubleRow,
                    )
                else:
                    for tt in range(2):
                        tloc = 2 * c + tt
                        nc.tensor.matmul(
                            acc[:, qt_, :],
                            ebb[:, tloc * 512 + qt_ * 128 :][:, :128],
                            vb[:, h, tloc, :],
                            start=(first and qt_ == 0 and tt == 0),
                            stop=(last and qt_ == RT - 1 and tt == 1),
                        )

        def emit_head_final(h):
            if OLD_HF:
                acc = accs[h]
                rec = spool.tile([128, RT], F32, name=f"rec{h}", tag="stat")
                nc.vector.reciprocal(rec[:], acc[:, :, 64:65])
                for qt_ in range(RT):
                    nc.vector.scalar_tensor_tensor(
                        o1v[:, qt_, h * 64 : (h + 1) * 64],
                        acc[:, qt_, 0:64],
                        rec[:, qt_ : qt_ + 1],
                        xbv[:, qt_, h * 64 : (h + 1) * 64],
                        Alu.mult,
                        Alu.add,
                    )
                return
            # wholesale acc drain (DVE) -> reciprocal (DVE) -> Pool scale;
            # the +x residual is added per-row-tile later (DVE 2x, all-SBUF)
            acc = accs[h]
            accS = accspool.tile([128, RT, 68], F32, name=f"accS{h}", tag="accs")
            nc.vector.tensor_copy(accS[:], acc[:])
            rec = spool.tile([128, RT], F32, name=f"rec{h}", tag="stat")
            nc.vector.reciprocal(rec[:], accS[:, :, 64:65])
            for qt_ in range(RT):
                nc.gpsimd.tensor_scalar(
                    o1v[:, qt_, h * 64 : (h + 1) * 64],
                    accS[:, qt_, 0:64],
                    rec[:, qt_ : qt_ + 1],
                    None,
                    Alu.mult,
                )

        # up-front: what gates head 0, engines in parallel; V units start
        # as soon as wv8+enc slices land (DMA order: wv8 early, enc chunk 1
        # first since the head loop leads with chunk 2 = lk tiles 4,5)
        A, V = nc.scalar, nc.vector
        emit_qt_unit(0, V)
        emit_kt_unit(0, 1, A)
        emit_v_unit(0, V)
        emit_v_unit(1, A)

        # chunk emission order per head: ACT chunks lead, hack (DVE) chunks
        # interleave so both drain engines stay fed from a 2-slot sc buffer
        ORDER = [2, 3, 0, 4, 5, 1, 6, 7] if NHACK == 2 else list(range(NCH))
        FIRST_C, LAST_C = ORDER[0], ORDER[-1]
        # just-in-time unit emissions: (head, slot) -> [thunks]
        JIT = {
            (0, 0): [lambda: emit_kt_unit(0, 0, V)],
            (0, 1): [lambda: emit_v_unit(2, V)],
            (0, 2): [lambda: emit_kt_unit(0, 2, A), lambda: emit_v_unit(3, V)],
            (0, 4): [lambda: emit_v_unit(4, V), lambda: emit_v_unit(5, A)],
            (0, 5): [lambda: emit_kt_unit(0, 3, V)],
            (0, 6): [lambda: emit_v_unit(6, A)],
            (0, 7): [lambda: emit_v_unit(7, V)],
            (1, 0): [lambda: emit_v_unit(8, A), lambda: emit_v_unit(9, V)],
            (1, 1): [lambda: emit_v_unit(10, V)],
            (1, 2): [lambda: emit_v_unit(11, V)],
            (1, 3): [lambda: emit_v_unit(12, A)],
            (1, 4): [lambda: emit_v_unit(13, V)],
            (1, 5): [lambda: emit_v_unit(14, A), lambda: emit_v_unit(15, V)],
            (1, 6): [lambda: emit_kt_unit(1, 1, V), lambda: emit_qt_unit(1, A)],
            (1, 7): [lambda: emit_kt_unit(1, 0, V)],
            (2, 1): [lambda: emit_kt_unit(1, 2, V)],
            (2, 2): [lambda: emit_kt_unit(1, 3, V)],
            (3, 0): [lambda: emit_kt_unit(2, 1, V)],
            (3, 1): [lambda: emit_qt_unit(2, V)],
            (3, 2): [lambda: emit_kt_unit(2, 0, V)],
            (3, 4): [lambda: emit_kt_unit(2, 2, V)],
            (3, 6): [lambda: emit_kt_unit(2, 3, V)],
            (5, 0): [lambda: emit_kt_unit(3, 1, V)],
            (5, 1): [lambda: emit_qt_unit(3, V)],
            (5, 2): [lambda: emit_kt_unit(3, 0, V)],
            (5, 4): [lambda: emit_kt_unit(3, 2, V)],
            (5, 6): [lambda: emit_kt_unit(3, 3, V)],
        }

        pending = []
        for h in range(H):
            pr, off = h // 2, 64 * (h % 2)
            e8 = e8s[h % 2]
            ebb = ebbs[h % 2]
            ktv = kt8[pr][:].rearrange("p (n w) -> p n w", w=128)
            accs[h] = pB.tile([128, RT, 68], F32, name=f"acc{h}", tag="pB")
            for slot, c in enumerate(ORDER):
                for jit in JIT.pop((h, slot), ()):
                    jit()
                sc = pA.tile([128, 1024], F32, name=f"sc{h}_{c}", tag="pA")
                for tt in range(2):
                    t = 2 * c + tt
                    nc.tensor.matmul(
                        sc[:, tt * 512 : (tt + 1) * 512],
                        ktv[off : off + 64, t : LT + 1 : LT - t, :],
                        qt8v[off : off + 64, pr : DT + 1 : DT - pr, :],
                        start=True,
                        stop=True,
                        perf_mode=DoubleRow,
                        tile_position=(off, 0),
                    )
                if not is_hack(h, c):
                    nc.scalar.activation(
                        e8[:, (c - NHACK) * 1024 : (c - NHACK + 1) * 1024],
                        sc[:],
                        mybir.ActivationFunctionType.Exp,
                        bias=moff_col[:, 0:1],
                        scale=2.0**-9,
                    )
                else:
                    nc.vector.tensor_scalar(
                        ebb[:, c * 1024 : (c + 1) * 1024].bitcast(I16),
                        sc[:],
                        HACK_MULT,
                        HACK_BIAS,
                        Alu.mult,
                        Alu.add,
                    )
                pending.append((h, c))
                if len(pending) > 7:
                    ph_, pc_ = pending.pop(0)
                    emit_attnv(ph_, pc_)
                    if pc_ == NCH - 1:
                        emit_head_final(ph_)
        for ph_, pc_ in pending:
            emit_attnv(ph_, pc_)
            if pc_ == NCH - 1:
                emit_head_final(ph_)

        # ---- LN1 -> out1 (bf16); wave-emitted so row-tiles pipeline ----
        out1b = ob1pool.tile([128, RT * D], BF16, name="out1b", tag="out1b")
        ob1v = out1b[:].rearrange("p (r d) -> p r d", d=D)

        def layer_norm(dst, src, name, gain_bc=None, bias_bc=None):
            bn6 = spool.tile([128, 6], F32, name=f"bn6{name}", tag="stat")
            nc.vector.bn_stats(bn6[:], src)
            mv = spool.tile([128, 2], F32, name=f"mv{name}", tag="stat")
            nc.vector.bn_aggr(mv[:], bn6[:])
            std = spool.tile([128, 1], F32, name=f"std{name}", tag="stat")
            nc.scalar.activation(
                std[:], mv[:, 1:2], mybir.ActivationFunctionType.Sqrt,
                bias=eps_col[:, 0:1],
            )
            rstd = spool.tile([128, 1], F32, name=f"rstd{name}", tag="stat")
            nc.vector.reciprocal(rstd[:], std[:])
            nc.gpsimd.tensor_scalar(
                dst, src, mv[:, 0:1], rstd[:, 0:1], Alu.subtract, Alu.mult
            )
            if gain_bc is not None:
                nc.gpsimd.tensor_tensor(dst, dst, gain_bc[:], Alu.mult)
                nc.gpsimd.tensor_tensor(dst, dst, bias_bc[:], Alu.add)

        bn6s, mvs, stds, rstds = [], [], [], []
        if not OLD_HF:
            for qt_ in range(RT):
                nc.vector.tensor_tensor(
                    o1v[:, qt_, :], o1v[:, qt_, :], xbv[:, qt_, :], Alu.add
                )
        for qt_ in range(RT):
            bn6 = spool.tile([128, 6], F32, name=f"bn6l1_{qt_}", tag="stat")
            nc.vector.bn_stats(bn6[:], o1v[:, qt_, :])
            bn6s.append(bn6)
        for qt_ in range(RT):
            mv = spool.tile([128, 2], F32, name=f"mvl1_{qt_}", tag="stat")
            nc.vector.bn_aggr(mv[:], bn6s[qt_][:])
            mvs.append(mv)
        for qt_ in range(RT):
            std = spool.tile([128, 1], F32, name=f"stdl1_{qt_}", tag="stat")
            nc.scalar.activation(
                std[:], mvs[qt_][:, 1:2], mybir.ActivationFunctionType.Sqrt,
                bias=eps_col[:, 0:1],
            )
            stds.append(std)
        for qt_ in range(RT):
            rstd = spool.tile([128, 1], F32, name=f"rstdl1_{qt_}", tag="stat")
            nc.vector.reciprocal(rstd[:], stds[qt_][:])
            rstds.append(rstd)
        for qt_ in range(RT):
            nc.vector.tensor_scalar(
                ob1v[:, qt_, :], o1v[:, qt_, :], mvs[qt_][:, 0:1],
                rstds[qt_][:, 0:1], Alu.subtract, Alu.mult,
            )

        # ---- out1^T (PE transpose via bf16 psum); rt-outer so each row
        # tile transposes right after its LN1 apply ----
        ident = cpool.tile([128, 128], F32)
        masks.make_identity(nc, ident[:])
        identb = cpool.tile([128, 128], BF16)
        nc.vector.tensor_copy(identb[:], ident[:])
        o1t = o1tpool.tile([128, DT * ROWS], F8, name="o1t", tag="o1t")
        o1tv = o1t[:].rearrange("p (n w) -> p n w", w=ROWS)
        pts01 = pB.tile([128, 2, ROWS], BF16, name="po1t01", tag="pB")
        pts2 = pC.tile([128, ROWS], BF16, name="po1t2", tag="pC")
        pts3 = pC.tile([128, ROWS], BF16, name="po1t3", tag="pC")

        def pt_ap(dt_):
            return (pts01[:, 0, :], pts01[:, 1, :], pts2[:], pts3[:])[dt_]

        for rt_ in range(RT):
            for dt_ in range(DT):
                nc.tensor.matmul(
                    pt_ap(dt_)[:, rt_ * 128 : (rt_ + 1) * 128],
                    ob1v[:, rt_, dt_ * 128 : (dt_ + 1) * 128],
                    identb[:],
                    is_transpose=True,
                    start=(rt_ == 0),
                    stop=(rt_ == RT - 1),
                )
        for dt_ in range(DT):
            # out1 (bf16 psum) -> fp8 x2^2 for the DoubleRow FFN1
            if dt_ < 2:
                nc.scalar.mul(o1tv[:, dt_, :], pt_ap(dt_), 4.0)
            else:
                nc.vector.tensor_scalar(o1tv[:, dt_, :], pt_ap(dt_), 4.0, None, Alu.mult)

        # ---- FFN (fp8 DR FFN1, bf16 FFN2), two row-half passes ----
        h1t = h1pool.tile([128, FT * ROWS], BF16, name="h1t", tag="h1t")
        h1v = h1t[:].rearrange("p (n w) -> p n w", w=ROWS)
        w1v = w1b[:].rearrange("p (n w) -> p n w", w=DFF)
        w2v = w2b[:].rearrange("p (n w) -> p n w", w=D)
        pff0 = pB.tile([128, D], F32, name="pff0", tag="pB")
        pff1 = pB.tile([128, D], F32, name="pff1", tag="pB")
        pff2 = pC.tile([128, D], F32, name="pff2", tag="pC")
        pff3 = pC.tile([128, D], F32, name="pff3", tag="pC")

        def pff_ap(rt_):
            return (pff0[:], pff1[:], pff2[:], pff3[:])[rt_]

        g2bc = be2bc = b2bc = None
        if apply_g2b2 or add_b2:
            def bcast(name, dram):
                row = cpool.tile([1, D], F32, name=f"{name}row")
                nc.sync.dma_start(row[:], dram[None, :])
                full = cpool.tile([128, D], F32, name=f"{name}bc")
                nc.gpsimd.partition_broadcast(full[:], row[:])
                return full

            g2bc = bcast("g2", g2_d)
            be2bc = bcast("be2", be2_d)
            b2bc = bcast("b2", b2_d)

        scr = [
            scrpool.tile([128, D], F32, name=f"scr{i}", tag="scr") for i in range(2)
        ]

        def emit_tail(rts):
            if add_b2:
                for rt_ in rts:
                    yt = ypool.tile([128, D], F32, name=f"y{rt_}", tag="y")
                    nc.vector.tensor_tensor(
                        yt[:], pff_ap(rt_), ob1v[:, rt_, :], Alu.add
                    )
                    nc.vector.tensor_tensor(yt[:], yt[:], b2bc[:], Alu.add)
                    layer_norm(
                        yt[:], yt[:], f"ln2_{rt_}",
                        gain_bc=g2bc if apply_g2b2 else None,
                        bias_bc=be2bc if apply_g2b2 else None,
                    )
                    nc.sync.dma_start(y_d[:, rt_ * D : (rt_ + 1) * D], yt[:])
                return
            yts, s1s, s2s, uss, mus = {}, {}, {}, {}, {}
            for rt_ in rts:
                yt = ypool.tile([128, D], F32, name=f"y{rt_}", tag="y")
                s1 = spool.tile([128, 1], F32, name=f"s1_{rt_}", tag="stat")
                nc.vector.scalar_tensor_tensor(
                    yt[:], pff_ap(rt_), 1.0, ob1v[:, rt_, :], Alu.mult, Alu.add,
                    accum_out=s1[:],
                )
                yts[rt_], s1s[rt_] = yt, s1
            for rt_ in rts:
                s2 = spool.tile([128, 1], F32, name=f"s2_{rt_}", tag="stat")
                nc.scalar.activation(
                    scr[rt_ % 2][:], yts[rt_][:],
                    mybir.ActivationFunctionType.Square, accum_out=s2[:],
                )
                s2s[rt_] = s2
            for rt_ in rts:
                # var = (s2 - s1^2/D)/D; std = sqrt(var + eps)
                u = spool.tile([128, 1], F32, name=f"u{rt_}", tag="stat")
                nc.vector.tensor_tensor(u[:], s1s[rt_][:], s1s[rt_][:], Alu.mult)
                nc.vector.tensor_scalar(u[:], u[:], 1.0 / D, None, Alu.mult)
                nc.vector.tensor_tensor(u[:], s2s[rt_][:], u[:], Alu.subtract)
                uss[rt_] = u
                mu = spool.tile([128, 1], F32, name=f"mu{rt_}", tag="stat")
                nc.vector.tensor_scalar(mu[:], s1s[rt_][:], 1.0 / D, None, Alu.mult)
                mus[rt_] = mu
            stds2 = {}
            for rt_ in rts:
                std = spool.tile([128, 1], F32, name=f"stdy{rt_}", tag="stat")
                nc.scalar.activation(
                    std[:], uss[rt_][:], mybir.ActivationFunctionType.Sqrt,
                    bias=eps_col[:, 0:1], scale=1.0 / D,
                )
                stds2[rt_] = std
            for rt_ in rts:
                rstd = spool.tile([128, 1], F32, name=f"rstdy{rt_}", tag="stat")
                nc.vector.reciprocal(rstd[:], stds2[rt_][:])
                nc.vector.tensor_scalar(
                    yts[rt_][:], yts[rt_][:], mus[rt_][:, 0:1],
                    rstd[:, 0:1], Alu.subtract, Alu.mult,
                )
                if apply_g2b2:
                    nc.vector.tensor_tensor(yts[rt_][:], yts[rt_][:], g2bc[:], Alu.mult)
                    nc.vector.tensor_tensor(yts[rt_][:], yts[rt_][:], be2bc[:], Alu.add)
                nc.sync.dma_start(y_d[:, rt_ * D : (rt_ + 1) * D], yts[rt_][:])

        HALF = ROWS // 2  # 256 rows per pass
        for half in range(2):
            r0 = half * HALF
            rts = [2 * half, 2 * half + 1]

            def emit_ffn2(s):
                for rt_ in rts:
                    nc.tensor.matmul(
                        pff_ap(rt_),
                        h1v[:, s, rt_ * 128 : (rt_ + 1) * 128],
                        w2v[:, s, :],
                        start=(s == 0),
                        stop=(s == FT - 1),
                    )

            for s in range(FT):
                ph = pA.tile([128, HALF], F32, name=f"ph{half}_{s}", tag="pA")
                for j in range(0, DT, 2):
                    nc.tensor.matmul(
                        ph[:],
                        w1v[:, j : j + 2, s * 128 : (s + 1) * 128],
                        o1tv[:, j : j + 2, r0 : r0 + HALF],
                        start=(j == 0),
                        stop=(j == DT - 2),
                        perf_mode=DoubleRow,
                    )
                # psum = 2^9 h1pre; h1 = relu(h1pre + b1) in bf16
                nc.scalar.activation(
                    h1v[:, s, r0 : r0 + HALF], ph[:],
                    mybir.ActivationFunctionType.Relu, bias=b1c[:, s : s + 1],
                    scale=2.0**-9,
                )
                if s > 1:
                    emit_ffn2(s - 2)
            emit_ffn2(FT - 2)
            emit_ffn2(FT - 1)
            emit_tail(rts)

    nc.compile()
    return nc


_CACHED = {}


def _get_nc(apply_g2b2: bool = False, add_b2: bool = False):
    key = (apply_g2b2, add_b2)
    if key not in _CACHED:
        _CACHED[key] = build_program(*key)
    return _CACHED[key]


def _f8(x, scale_pow):
    return (np.asarray(x, np.float32) * (2.0**scale_pow)).astype(F8NP)


def _ktile_rows(a):
    """[K, M] -> [128, (K//128)*M]: out[p, j*M + m] = a[j*128 + p, m]."""
    K, M = a.shape
    return np.ascontiguousarray(
        a.reshape(K // 128, 128, M).transpose(1, 0, 2).reshape(128, -1)
    )


def kernel(**inputs) -> np.ndarray:
    x = np.asarray(inputs["inputs"], dtype=np.float32)
    enc = np.asarray(inputs["encoder_x"], dtype=np.float32)
    assert x.shape == (B, LQ, D) and enc.shape == (B, LK, D)
    assert int(np.asarray(inputs["n_heads"])) == H

    Wq = np.asarray(inputs["Wq"], np.float32)
    Wk = np.asarray(inputs["Wk"], np.float32)
    Wv = np.asarray(inputs["Wv"], np.float32)
    g1 = np.asarray(inputs["ln1_g"], np.float64)
    be1 = np.asarray(inputs["ln1_b"], np.float64)
    w1_raw = np.asarray(inputs["W1"], np.float64)
    w1_eff = (g1[:, None] * w1_raw).astype(np.float32)
    b1_eff = (np.asarray(inputs["b1"], np.float64) + be1 @ w1_raw).astype(np.float32)
    W2 = np.asarray(inputs["W2"], np.float32)
    b2 = np.asarray(inputs["b2"], np.float32)
    g2 = np.asarray(inputs["ln2_g"], np.float32)
    be2 = np.asarray(inputs["ln2_b"], np.float32)

    apply_g2b2 = not (np.allclose(g2, 1.0) and np.allclose(be2, 0.0))
    add_b2 = not np.allclose(b2, 0.0)
    nc = _get_nc(apply_g2b2, add_b2)

    shared = {
        "wq8": _ktile_rows(_f8(Wq, 5)),
        "wk8": _ktile_rows(_f8(Wk, 5)),
        "wv8": _ktile_rows(_f8(Wv, 5)),
        "w1b": _ktile_rows(_f8(w1_eff, 7)),
        "w2b": _ktile_rows(W2.astype(BF16NP)),
        "b1c": np.ascontiguousarray(_ktile_rows(b1_eff[:, None]).astype(np.float32)),
        "g2": np.ascontiguousarray(g2),
        "be2": np.ascontiguousarray(be2),
        "b2": np.ascontiguousarray(b2),
    }
    xf = x.reshape(B * LQ, D)
    in_maps = []
    for c in range(N_CORES):
        b = c // (N_CORES // B)
        xs = xf[c * ROWS : (c + 1) * ROWS]
        m = dict(shared)
        m["xt8"] = _ktile_rows(_f8(np.ascontiguousarray(xs.T), 4))
        m["xb"] = _ktile_rows(xs.astype(BF16NP))
        m["enct8"] = _ktile_rows(_f8(np.ascontiguousarray(enc[b].T), 4))
        in_maps.append(m)

    res = run_bass_kernel_spmd(nc, in_maps, core_ids=list(range(N_CORES)))
    out = np.empty((B * LQ, D), np.float32)
    for c in range(N_CORES):
        yc = res.results[c]["y"].reshape(128, RT, D).transpose(1, 0, 2).reshape(ROWS, D)
        out[c * ROWS : (c + 1) * ROWS] = yc
    return out.reshape(B, LQ, D)


# revision 15
# speedup vs baseline: 1.0878x; 1.0605x over previous
"""Trainium2 Bass kernel for a cross-attention decoder block.

Shapes (hardcoded): B=2, LQ=LK=2048, D=512, H=8 heads (hd=64), DFF=2048.

    q = x @ Wq; k = enc @ Wk; v = enc @ Wv            (per batch)
    attn = softmax(q k^T / sqrt(hd)); o = attn v
    out1 = LayerNorm(o + x)
    y = LayerNorm(relu(out1 @ W1 + b1) @ W2 + b2 + out1)

Sharding: row-parallel over the 4096 flattened query rows; 8 cores x 512 rows.
Cores 0-3 take batch 0, cores 4-7 batch 1. Each core computes its batch's full
K/V locally (replicated within the 4-core group) -- no collectives.

Engine budget (TimelineSim cost model): the kernel is PSUM-drain bound --
only ACT (0.833 ns/el) and DVE (1.042 ns/el) can read PSUM; Pool/GPSIMD has
no PSUM port.  Assignment:
  - ACT: softmax exp for chunks 2-7 of every head (fp8 out, Exp activation).
  - DVE: exp-bits hack for chunks 0-1 of every head (one 1024-wide two-op
    tensor_scalar: bits = round(s1*qk + s2) as int16 == bf16 e^(S-3)*(1+-2%)),
    all projection emissions (KT/V/qT psum->fp8, paired 1024-wide), attention
    accumulator drains, LN stats/applies (all-SBUF tensor_scalar runs at 2x).
  - Pool: per-head softmax-normalize + residual (SBUF-side stt), memsets.
  - PE: fp8 DoubleRow matmuls (projections, scores, attnV for ACT chunks,
    FFN1), bf16 (attnV for hack chunks, FFN2, transposes).

Numerics (validated on hw, rel ~1.2e-2 vs the 2e-2 gate):
  - host pre-quantizes operands: x^T/enc^T/Wq/Wk/Wv in fp8e4m3 (pow-2 scales),
    W1 fp8, W2/x in bf16; transposes done on the host for free.
  - KT slabs are uniformly scaled (2^3 k^T); the non-pow2 exp-hack slope is
    applied inside the DVE tensor_scalar (mult op), not folded into KT.
  - attn@V: fp8 DoubleRow for ACT chunks, bf16 for hack chunks, accumulated
    into one PSUM tile; a 16.0 "ones" column yields the softmax denominator.
"""

import sys

sys.path.insert(0, "/opt/trn_rl_repo")

from contextlib import ExitStack

import numpy as np
import ml_dtypes

import concourse.bacc as bacc
import concourse.bass as bass
import concourse.mybir as mybir
from concourse import masks, tile
from concourse.bass_utils import run_bass_kernel_spmd

F32 = mybir.dt.float32
BF16 = mybir.dt.bfloat16
F8 = mybir.dt.float8e4
I16 = mybir.dt.int16
F8NP = ml_dtypes.float8_e4m3fn
BF16NP = ml_dtypes.bfloat16

B, LQ, LK, D, H, DFF = 2, 2048, 2048, 512, 8, 2048
HD = D // H  # 64
N_CORES = 8
ROWS = B * LQ // N_CORES  # 512 query rows per core
RT = ROWS // 128  # 4 row tiles
DT = D // 128  # 4 d tiles
LT = LK // 128  # 16 lk tiles
FT = DFF // 128  # 16 dff tiles
NCH = LT // 2  # 8 chunks per head (2 lk tiles each)
import os
NHACK = int(os.environ.get("KNHACK", "2"))  # chunks 0..NHACK-1 per head: DVE exp-bits hack
OLD_HF = bool(int(os.environ.get("KOLDHF", "0")))  # baseline head_final path
EPS = 1e-5
LN2E = float(np.log(2.0))

EOFF = 3.0  # e' = exp(S - EOFF)
# psum scores = 2^9 * S (S = qk/sqrt(hd)); bits = HACK_MULT*psum + HACK_BIAS
HACK_MULT = (128.0 / LN2E) / 512.0
HACK_BIAS = 16256.0 - 7.0 - EOFF * 128.0 / LN2E
KTW = (LT + 1) * 128  # KT slab width incl. the zero k-tile tail

DoubleRow = mybir.MatmulPerfMode.DoubleRow
Alu = mybir.AluOpType


def build_program(apply_g2b2: bool, add_b2: bool, b1_zero: bool = True) -> bass.Bass:
    nc = bacc.Bacc(None, target_bir_lowering=False, debug=False)

    xt8_d = nc.dram_tensor("xt8", [128, DT * ROWS], F8, kind="ExternalInput")
    xb_d = nc.dram_tensor("xb", [128, RT * D], BF16, kind="ExternalInput")
    enct8_d = nc.dram_tensor("enct8", [128, DT * LK], F8, kind="ExternalInput")
    wq8_d = nc.dram_tensor("wq8", [128, DT * D], F8, kind="ExternalInput")
    wk8_d = nc.dram_tensor("wk8", [128, DT * D], F8, kind="ExternalInput")
    wv8_d = nc.dram_tensor("wv8", [128, DT * D], F8, kind="ExternalInput")
    w1b_d = nc.dram_tensor("w1b", [128, DT * DFF], F8, kind="ExternalInput")
    w2b_d = nc.dram_tensor("w2b", [128, FT * D], F8, kind="ExternalInput")
    b1c_d = nc.dram_tensor("b1c", [128, FT], F32, kind="ExternalInput")
    g2_d = nc.dram_tensor("g2", [D], F32, kind="ExternalInput")
    be2_d = nc.dram_tensor("be2", [D], F32, kind="ExternalInput")
    b2_d = nc.dram_tensor("b2", [D], F32, kind="ExternalInput")
    y_d = nc.dram_tensor("y", [128, RT * D], F32, kind="ExternalOutput")

    with ExitStack() as ctx:
        tc = ctx.enter_context(tile.TileContext(nc))
        cpool = ctx.enter_context(tc.tile_pool(name="const", bufs=1))
        wpool = ctx.enter_context(tc.tile_pool(name="w8", bufs=4))
        encpool = ctx.enter_context(tc.tile_pool(name="enc8", bufs=1))
        w1pool = ctx.enter_context(tc.tile_pool(name="w1b", bufs=1))
        w2pool = ctx.enter_context(tc.tile_pool(name="w2b", bufs=1))
        xbpool = ctx.enter_context(tc.tile_pool(name="xb", bufs=1))
        qtpool = ctx.enter_context(tc.tile_pool(name="qt8", bufs=1))
        ktpool = ctx.enter_context(tc.tile_pool(name="kt8", bufs=4))
        vpool = ctx.enter_context(tc.tile_pool(name="v8", bufs=1))
        vbpool = ctx.enter_context(tc.tile_pool(name="vb", bufs=1))
        e8pool = ctx.enter_context(tc.tile_pool(name="e8", bufs=2))
        ebpool = ctx.enter_context(tc.tile_pool(name="ebb", bufs=2))
        o1pool = ctx.enter_context(tc.tile_pool(name="o1", bufs=1))
        accspool = ctx.enter_context(tc.tile_pool(name="accs", bufs=2))
        ob1pool = ctx.enter_context(tc.tile_pool(name="out1b", bufs=1))
        o1tpool = ctx.enter_context(tc.tile_pool(name="o1t", bufs=1))
        h1pool = ctx.enter_context(tc.tile_pool(name="h1t", bufs=1))
        ypool = ctx.enter_context(tc.tile_pool(name="y", bufs=4))
        scrpool = ctx.enter_context(tc.tile_pool(name="scr", bufs=2))
        spool = ctx.enter_context(tc.tile_pool(name="stat", bufs=16))
        # PSUM: pA = 2 slots x 2 banks (sc chunks; later ffn1/transpose),
        # pB = 2 x 1 bank (attnV accums -> pff01/pts01),
        # pC = 2 x 1 bank (projection 512-wide units -> pff23/pts23).
        pA = ctx.enter_context(tc.tile_pool(name="pA", bufs=2, space="PSUM"))
        pB = ctx.enter_context(tc.tile_pool(name="pB", bufs=2, space="PSUM"))
        pC = ctx.enter_context(tc.tile_pool(name="pC", bufs=2, space="PSUM"))

        # ---- PE warmup through the p-state ramp while the first DMAs land ----
        wsrc = cpool.tile([128, 128], BF16)
        nc.gpsimd.memset(wsrc[:], 0.0)
        for i in range(16):
            wp = pA.tile([128, 128], F32, name=f"warm{i}", tag="pA")
            nc.tensor.matmul(wp[:], wsrc[:], wsrc[:], start=True, stop=True)

        # ---- constants ----
        eps_col = cpool.tile([128, 1], F32)
        nc.gpsimd.memset(eps_col[:], EPS)
        moff_col = cpool.tile([128, 1], F32)
        nc.gpsimd.memset(moff_col[:], -EOFF)

        # ---- input loads (first-needed first) ----
        def load(pool_, name, dram, cols, dt_):
            t = pool_.tile([128, cols], dt_, name=name, tag=name)
            nc.sync.dma_start(t[:], dram[:, :])
            return t

        xt8 = load(wpool, "xt8", xt8_d, DT * ROWS, F8)
        wq8 = load(wpool, "wq8", wq8_d, DT * D, F8)
        wk8 = load(wpool, "wk8", wk8_d, DT * D, F8)
        wv8 = load(wpool, "wv8", wv8_d, DT * D, F8)
        enct8 = encpool.tile([128, DT * LK], F8, name="enct8", tag="enct8")
        encdv = enct8_d[:, :].rearrange("p (n w) -> p n w", w=LK)
        enctv_ = enct8[:].rearrange("p (n w) -> p n w", w=LK)
        for k in (1, 0, 2, 3):
            nc.sync.dma_start(
                enctv_[:, :, k * 512 : (k + 1) * 512],
                encdv[:, :, k * 512 : (k + 1) * 512],
            )
        xb = load(xbpool, "xb", xb_d, RT * D, BF16)
        b1c = load(cpool, "b1c", b1c_d, FT, F32)
        w1b = load(w1pool, "w1b", w1b_d, DT * DFF, F8)
        w2b = load(w2pool, "w2b", w2b_d, FT * D, F8)

        xt8v = xt8[:].rearrange("p (n w) -> p n w", w=ROWS)
        wq8v = wq8[:].rearrange("p (n w) -> p n w", w=D)
        wk8v = wk8[:].rearrange("p (n w) -> p n w", w=D)
        wv8v = wv8[:].rearrange("p (n w) -> p n w", w=D)
        enct8v = enct8[:].rearrange("p (n w) -> p n w", w=LK)
        xbv = xb[:].rearrange("p (r d) -> p r d", d=D)

        # ---- qT projection: 2 pairs [128,1024], fp8 DR matmuls, DVE drain ----
        qt8 = qtpool.tile([128, DT * ROWS + ROWS], F8, name="qt8", tag="qt8")
        nc.gpsimd.memset(qt8[:, DT * ROWS :], 0.0)
        qt8v = qt8[:].rearrange("p (n w) -> p n w", w=ROWS)

        def drain(eng, dst, psum, scale):
            if eng is nc.scalar:
                nc.scalar.mul(dst, psum, scale)
            else:
                eng.tensor_scalar(dst, psum, scale, None, Alu.mult)

        def emit_qt_unit(s, eng=None):
            pq = pC.tile([128, 512], F32, name=f"pq{s}", tag="pC")
            for j in range(0, DT, 2):
                nc.tensor.matmul(
                    pq[:],
                    wq8v[:, j : j + 2, s * 128 : (s + 1) * 128],
                    xt8v[:, j : j + 2, :],
                    start=(j == 0),
                    stop=(j == DT - 2),
                    perf_mode=DoubleRow,
                )
            drain(eng or nc.vector, qt8[:, s * ROWS : (s + 1) * ROWS], pq[:], 2.0**-6)

        # ---- KT slabs (uniform 2^-6 scale; zero k-tile tail) ----
        kt8 = [
            ktpool.tile([128, KTW], F8, name=f"kt8_{s}", tag="kt8") for s in range(DT)
        ]
        for s in range(DT):
            nc.gpsimd.memset(kt8[s][:, LT * 128 :], 0.0)

        def emit_kt_unit(s, cb, eng=None):
            """KT slab s, lk columns [512*cb, 512*(cb+1))."""
            pk = pC.tile([128, 512], F32, name=f"pk{s}_{cb}", tag="pC")
            for j in range(0, DT, 2):
                nc.tensor.matmul(
                    pk[:],
                    wk8v[:, j : j + 2, s * 128 : (s + 1) * 128],
                    enct8v[:, j : j + 2, cb * 512 : (cb + 1) * 512],
                    start=(j == 0),
                    stop=(j == DT - 2),
                    perf_mode=DoubleRow,
                )
            drain(eng or nc.vector, kt8[s][:, cb * 512 : (cb + 1) * 512], pk[:], 2.0**-6)

        # ---- V layout: chunks 0-1 (hack) -> vb bf16; chunks 2-7 -> v8 fp8;
        # col 64 = 16.0 softmax-denominator column ----
        v8 = vpool.tile([128, H, NCH - NHACK, 2, 68], F8, name="v8", tag="v8")
        v8f = v8[:].rearrange("p a b c d -> p (a b c) d")
        nc.gpsimd.memset(v8f[:, :, 64:65], 16.0)
        nc.gpsimd.memset(v8f[:, :, 65:68], 0.0)
        if NHACK:
            vb = vbpool.tile([128, H, 2 * NHACK, 68], BF16, name="vb", tag="vb")
            vbf = vb[:].rearrange("p a b c -> p (a b) c")
            nc.gpsimd.memset(vbf[:, :, 64:65], 16.0)
            nc.gpsimd.memset(vbf[:, :, 65:68], 0.0)

        def emit_v_unit(t, eng=None):
            """lk tile t -> one [128,512] psum, one drain."""
            pv = pC.tile([128, 512], F32, name=f"pv{t}", tag="pC")
            for j in range(0, DT, 2):
                nc.tensor.matmul(
                    pv[:],
                    enct8v[:, j : j + 2, t * 128 : (t + 1) * 128],
                    wv8v[:, j : j + 2, :],
                    start=(j == 0),
                    stop=(j == DT - 2),
                    perf_mode=DoubleRow,
                )
            pvh = pv[:].rearrange("p (h d) -> p h d", h=H)
            if t >= 2 * NHACK:
                drain(eng or nc.vector, v8[:, :, t // 2 - NHACK, t % 2, 0:64], pvh, 2.0**-5)
            else:
                drain(eng or nc.vector, vb[:, :, t, 0:64], pvh, 2.0**-5)

        # ---- attention (software pipelined, projections interleaved) ----
        # o1 pre-filled with the x residual (Pool); head finals accumulate
        # their normalized attention output on top
        o1 = o1pool.tile([128, RT * D], F32, name="o1", tag="o1")
        o1v = o1[:].rearrange("p (r d) -> p r d", d=D)
        nc.gpsimd.tensor_copy(o1[:], xb[:])
        e8s = [
            e8pool.tile([128, (NCH - NHACK) * 1024], F8, name=f"e8_{i}", tag="e8")
            for i in range(2)
        ]
        ebbs = [
            ebpool.tile([128, max(NHACK, 1) * 1024], BF16, name=f"ebb{i}", tag="ebb")
            for i in range(2)
        ]
        accs = [None] * H

        def is_hack(h, c):
            return c < NHACK

        def emit_attnv(h, c):
            e8 = e8s[h % 2]
            ebb = ebbs[h % 2]
            e8v = e8[:].rearrange("p (t q) -> p t q", q=512)
            acc = accs[h]
            first = c == FIRST_C
            last = c == LAST_C
            for qt_ in range(RT):
                if not is_hack(h, c):
                    cc = c - NHACK
                    nc.tensor.matmul(
                        acc[:, qt_, :],
                        e8v[:, 2 * cc : 2 * cc + 2, qt_ * 128 : (qt_ + 1) * 128],
                        v8[:, h, cc, :, :],
                        start=(first and qt_ == 0),
                        stop=(last and qt_ == RT - 1),
                        perf_mode=DoubleRow,
                    )
                else:
                    for tt in range(2):
                        tloc = 2 * c + tt
                        nc.tensor.matmul(
                            acc[:, qt_, :],
                            ebb[:, tloc * 512 + qt_ * 128 :][:, :128],
                            vb[:, h, tloc, :],
                            start=(first and qt_ == 0 and tt == 0),
                            stop=(last and qt_ == RT - 1 and tt == 1),
                        )

        def emit_head_final(h):
            if OLD_HF:
                acc = accs[h]
                rec = spool.tile([128, RT], F32, name=f"rec{h}", tag="stat")
                nc.vector.reciprocal(rec[:], acc[:, :, 64:65])
                for qt_ in range(RT):
                    nc.vector.scalar_tensor_tensor(
                        o1v[:, qt_, h * 64 : (h + 1) * 64],
                        acc[:, qt_, 0:64],
                        rec[:, qt_ : qt_ + 1],
                        xbv[:, qt_, h * 64 : (h + 1) * 64],
                        Alu.mult,
                        Alu.add,
                    )
                return
            # wholesale acc drain (DVE) -> reciprocal (DVE) -> Pool scale;
            # the +x residual is added per-row-tile later (DVE 2x, all-SBUF)
            acc = accs[h]
            accS = accspool.tile([128, RT, 68], F32, name=f"accS{h}", tag="accs")
            nc.vector.tensor_copy(accS[:], acc[:])
            rec = spool.tile([128, RT], F32, name=f"rec{h}", tag="stat")
            nc.vector.reciprocal(rec[:], accS[:, :, 64:65])
            osc = accspool.tile([128, RT, 64], F32, name=f"osc{h}", tag="osc")
            for qt_ in range(RT):
                nc.gpsimd.tensor_scalar(
                    osc[:, qt_, :],
                    accS[:, qt_, 0:64],
                    rec[:, qt_ : qt_ + 1],
                    None,
                    Alu.mult,
                )
                nc.gpsimd.tensor_tensor(
                    o1v[:, qt_, h * 64 : (h + 1) * 64],
                    o1v[:, qt_, h * 64 : (h + 1) * 64],
                    osc[:, qt_, :],
                    Alu.add,
                )

        # up-front: what gates head 0, engines in parallel; V units start
        # as soon as wv8+enc slices land (DMA order: wv8 early, enc chunk 1
        # first since the head loop leads with chunk 2 = lk tiles 4,5)
        A, V = nc.scalar, nc.vector
        emit_qt_unit(0, V)
        emit_kt_unit(0, 1, A)
        emit_v_unit(0, V)
        emit_v_unit(1, A)

        # chunk emission order per head: ACT chunks lead, hack (DVE) chunks
        # interleave so both drain engines stay fed from a 2-slot sc buffer
        ORDER = [2, 3, 0, 4, 5, 1, 6, 7] if NHACK == 2 else list(range(NCH))
        FIRST_C, LAST_C = ORDER[0], ORDER[-1]
        # just-in-time unit emissions: (head, slot) -> [thunks]
        JIT = {
            (0, 0): [lambda: emit_kt_unit(0, 0, V)],
            (0, 1): [lambda: emit_v_unit(2, V)],
            (0, 2): [lambda: emit_kt_unit(0, 2, A), lambda: emit_v_unit(3, V)],
            (0, 4): [lambda: emit_v_unit(4, V), lambda: emit_v_unit(5, A)],
            (0, 5): [lambda: emit_kt_unit(0, 3, V)],
            (0, 6): [lambda: emit_v_unit(6, A)],
            (0, 7): [lambda: emit_v_unit(7, V)],
            (1, 0): [lambda: emit_v_unit(8, A), lambda: emit_v_unit(9, V)],
            (1, 1): [lambda: emit_v_unit(10, V)],
            (1, 2): [lambda: emit_v_unit(11, V)],
            (1, 3): [lambda: emit_v_unit(12, A)],
            (1, 4): [lambda: emit_v_unit(13, V)],
            (1, 5): [lambda: emit_v_unit(14, A), lambda: emit_v_unit(15, V)],
            (1, 6): [lambda: emit_kt_unit(1, 1, V), lambda: emit_qt_unit(1, A)],
            (1, 7): [lambda: emit_kt_unit(1, 0, V)],
            (2, 1): [lambda: emit_kt_unit(1, 2, V)],
            (2, 2): [lambda: emit_kt_unit(1, 3, V)],
            (3, 0): [lambda: emit_kt_unit(2, 1, V)],
            (3, 1): [lambda: emit_qt_unit(2, V)],
            (3, 2): [lambda: emit_kt_unit(2, 0, V)],
            (3, 4): [lambda: emit_kt_unit(2, 2, V)],
            (3, 6): [lambda: emit_kt_unit(2, 3, V)],
            (5, 0): [lambda: emit_kt_unit(3, 1, V)],
            (5, 1): [lambda: emit_qt_unit(3, V)],
            (5, 2): [lambda: emit_kt_unit(3, 0, V)],
            (5, 4): [lambda: emit_kt_unit(3, 2, V)],
            (5, 6): [lambda: emit_kt_unit(3, 3, V)],
        }

        pending = []
        for h in range(H):
            pr, off = h // 2, 64 * (h % 2)
            e8 = e8s[h % 2]
            ebb = ebbs[h % 2]
            ktv = kt8[pr][:].rearrange("p (n w) -> p n w", w=128)
            accs[h] = pB.tile([128, RT, 68], F32, name=f"acc{h}", tag="pB")
            for slot, c in enumerate(ORDER):
                for jit in JIT.pop((h, slot), ()):
                    jit()
                sc = pA.tile([128, 1024], F32, name=f"sc{h}_{c}", tag="pA")
                for tt in range(2):
                    t = 2 * c + tt
                    nc.tensor.matmul(
                        sc[:, tt * 512 : (tt + 1) * 512],
                        ktv[off : off + 64, t : LT + 1 : LT - t, :],
                        qt8v[off : off + 64, pr : DT + 1 : DT - pr, :],
                        start=True,
                        stop=True,
                        perf_mode=DoubleRow,
                        tile_position=(off, 0),
                    )
                if not is_hack(h, c):
                    nc.scalar.activation(
                        e8[:, (c - NHACK) * 1024 : (c - NHACK + 1) * 1024],
                        sc[:],
                        mybir.ActivationFunctionType.Exp,
                        bias=moff_col[:, 0:1],
                        scale=2.0**-9,
                    )
                else:
                    nc.vector.tensor_scalar(
                        ebb[:, c * 1024 : (c + 1) * 1024].bitcast(I16),
                        sc[:],
                        HACK_MULT,
                        HACK_BIAS,
                        Alu.mult,
                        Alu.add,
                    )
                pending.append((h, c))
                if len(pending) > 7:
                    ph_, pc_ = pending.pop(0)
                    emit_attnv(ph_, pc_)
                    if pc_ == NCH - 1:
                        emit_head_final(ph_)
        for ph_, pc_ in pending:
            emit_attnv(ph_, pc_)
            if pc_ == NCH - 1:
                emit_head_final(ph_)

        # DVE rsqrt via the bf16 bits trick + 2 Newton steps (keeps ACT on
        # the exp table set -- every set contains Copy/Relu/Square, so the
        # kernel never pays the 1.28us LoadActFuncSet switch for Sqrt)
        def rsqrt_cols(cols, name):
            n = len(cols)
            v4 = spool.tile([128, n], F32, name=f"v4{name}", tag="stat")
            for i, c in enumerate(cols):
                nc.vector.tensor_scalar(v4[:, i : i + 1], c, EPS, None, Alu.add)
            vb16 = spool.tile([128, n], BF16, name=f"vb16{name}", tag="stat")
            nc.vector.tensor_copy(vb16[:], v4[:])
            y0 = spool.tile([128, n], BF16, name=f"y0{name}", tag="stat")
            nc.vector.tensor_scalar(
                y0[:].bitcast(I16), vb16[:].bitcast(I16), -0.5, 24375.0,
                Alu.mult, Alu.add,
            )
            y = y0
            for it in range(2):
                t1 = spool.tile([128, n], F32, name=f"t1{name}{it}", tag="stat")
                nc.vector.tensor_tensor(t1[:], v4[:], y[:], Alu.mult)
                nc.vector.tensor_tensor(t1[:], t1[:], y[:], Alu.mult)
                nc.vector.tensor_scalar(t1[:], t1[:], -0.5, 1.5, Alu.mult, Alu.add)
                yn = spool.tile([128, n], F32, name=f"yn{name}{it}", tag="stat")
                nc.vector.tensor_tensor(yn[:], y[:], t1[:], Alu.mult)
                y = yn
            return y

        # ---- LN1 -> out1 (bf16); wave-emitted so row-tiles pipeline ----
        out1b = ob1pool.tile([128, RT * D], BF16, name="out1b", tag="out1b")
        ob1v = out1b[:].rearrange("p (r d) -> p r d", d=D)

        def layer_norm(dst, src, name, gain_bc=None, bias_bc=None):
            bn6 = spool.tile([128, 6], F32, name=f"bn6{name}", tag="stat")
            nc.vector.bn_stats(bn6[:], src)
            mv = spool.tile([128, 2], F32, name=f"mv{name}", tag="stat")
            nc.vector.bn_aggr(mv[:], bn6[:])
            std = spool.tile([128, 1], F32, name=f"std{name}", tag="stat")
            nc.scalar.activation(
                std[:], mv[:, 1:2], mybir.ActivationFunctionType.Sqrt,
                bias=eps_col[:, 0:1],
            )
            rstd = spool.tile([128, 1], F32, name=f"rstd{name}", tag="stat")
            nc.vector.reciprocal(rstd[:], std[:])
            nc.gpsimd.tensor_scalar(
                dst, src, mv[:, 0:1], rstd[:, 0:1], Alu.subtract, Alu.mult
            )
            if gain_bc is not None:
                nc.gpsimd.tensor_tensor(dst, dst, gain_bc[:], Alu.mult)
                nc.gpsimd.tensor_tensor(dst, dst, bias_bc[:], Alu.add)

        bn6s, mvs, rstds = [], [], []
        for qt_ in range(RT):
            bn6 = spool.tile([128, 6], F32, name=f"bn6l1_{qt_}", tag="stat")
            nc.vector.bn_stats(bn6[:], o1v[:, qt_, :])
            bn6s.append(bn6)
        for qt_ in range(RT):
            mv = spool.tile([128, 2], F32, name=f"mvl1_{qt_}", tag="stat")
            nc.vector.bn_aggr(mv[:], bn6s[qt_][:])
            mvs.append(mv)
        rstd4 = rsqrt_cols([mvs[qt_][:, 1:2] for qt_ in range(RT)], "l1")
        rstds = [rstd4[:, qt_ : qt_ + 1] for qt_ in range(RT)]
        for qt_ in range(RT):
            nc.vector.tensor_scalar(
                ob1v[:, qt_, :], o1v[:, qt_, :], mvs[qt_][:, 0:1],
                rstds[qt_][:, 0:1], Alu.subtract, Alu.mult,
            )

        # ---- out1^T (PE transpose via bf16 psum); rt-outer so each row
        # tile transposes right after its LN1 apply ----
        ident = cpool.tile([128, 128], F32)
        masks.make_identity(nc, ident[:])
        identb = cpool.tile([128, 128], BF16)
        nc.vector.tensor_copy(identb[:], ident[:])
        o1t = o1tpool.tile([128, DT * ROWS], F8, name="o1t", tag="o1t")
        o1tv = o1t[:].rearrange("p (n w) -> p n w", w=ROWS)
        pts01 = pB.tile([128, 2, ROWS], BF16, name="po1t01", tag="pB")
        pts2 = pC.tile([128, ROWS], BF16, name="po1t2", tag="pC")
        pts3 = pC.tile([128, ROWS], BF16, name="po1t3", tag="pC")

        def pt_ap(dt_):
            return (pts01[:, 0, :], pts01[:, 1, :], pts2[:], pts3[:])[dt_]

        for rt_ in range(RT):
            for dt_ in range(DT):
                nc.tensor.matmul(
                    pt_ap(dt_)[:, rt_ * 128 : (rt_ + 1) * 128],
                    ob1v[:, rt_, dt_ * 128 : (dt_ + 1) * 128],
                    identb[:],
                    is_transpose=True,
                    start=(rt_ == 0),
                    stop=(rt_ == RT - 1),
                )
        for dt_ in range(DT):
            # out1 (bf16 psum) -> fp8 x2^2 for the DoubleRow FFN1
            if dt_ < 2:
                nc.scalar.mul(o1tv[:, dt_, :], pt_ap(dt_), 4.0)
            else:
                nc.vector.tensor_scalar(o1tv[:, dt_, :], pt_ap(dt_), 4.0, None, Alu.mult)

        # ---- FFN: fp8 DoubleRow both matmuls.  h1 stored as 2^5*relu in
        # fp8; W2 host-quantized fp8 (2^5) -> ffn2 psum = 2^10 * ff ----
        h1t = h1pool.tile([128, FT * ROWS], F8, name="h1t", tag="h1t")
        h1v = h1t[:].rearrange("p (n w) -> p n w", w=ROWS)
        w1v = w1b[:].rearrange("p (n w) -> p n w", w=DFF)
        w2v = w2b[:].rearrange("p (n w) -> p n w", w=D)
        pff0 = pB.tile([128, D], F32, name="pff0", tag="pB")
        pff1 = pB.tile([128, D], F32, name="pff1", tag="pB")
        pff2 = pC.tile([128, D], F32, name="pff2", tag="pC")
        pff3 = pC.tile([128, D], F32, name="pff3", tag="pC")

        def pff_ap(rt_):
            return (pff0[:], pff1[:], pff2[:], pff3[:])[rt_]

        g2bc = be2bc = b2bc = None
        if apply_g2b2 or add_b2:
            def bcast(name, dram):
                row = cpool.tile([1, D], F32, name=f"{name}row")
                nc.sync.dma_start(row[:], dram[None, :])
                full = cpool.tile([128, D], F32, name=f"{name}bc")
                nc.gpsimd.partition_broadcast(full[:], row[:])
                return full

            g2bc = bcast("g2", g2_d)
            be2bc = bcast("be2", be2_d)
            b2bc = bcast("b2", b2_d)

        scr = [
            scrpool.tile([128, D], F32, name=f"scr{i}", tag="scr") for i in range(2)
        ]

        def emit_ffn2_pair(s):
            for rt_ in range(RT):
                nc.tensor.matmul(
                    pff_ap(rt_),
                    h1v[:, s : s + 2, rt_ * 128 : (rt_ + 1) * 128],
                    w2v[:, s : s + 2, :],
                    start=(s == 0),
                    stop=(s == FT - 2),
                    perf_mode=DoubleRow,
                )

        for s in range(FT):
            ph = pA.tile([128, ROWS], F32, name=f"ph{s}", tag="pA")
            for j in range(0, DT, 2):
                nc.tensor.matmul(
                    ph[:],
                    w1v[:, j : j + 2, s * 128 : (s + 1) * 128],
                    o1tv[:, j : j + 2, :],
                    start=(j == 0),
                    stop=(j == DT - 2),
                    perf_mode=DoubleRow,
                )
            # psum = 2^9 h1pre; h1_8 = relu(2^-4 psum + 2^5 b1)
            if b1_zero and s % 2 == 1:
                nc.vector.tensor_scalar(
                    h1v[:, s, :], ph[:], 2.0**-4, 0.0, Alu.mult, Alu.max
                )
            else:
                nc.scalar.activation(
                    h1v[:, s, :], ph[:],
                    mybir.ActivationFunctionType.Relu, bias=b1c[:, s : s + 1],
                    scale=2.0**-4,
                )
            if s >= 3 and s % 2 == 1:
                emit_ffn2_pair(s - 3)
        emit_ffn2_pair(FT - 2)

        # ---- residual + LN2 tail (single pass, DVE rsqrt) ----
        if add_b2:
            for rt_ in range(RT):
                yt = ypool.tile([128, D], F32, name=f"y{rt_}", tag="y")
                nc.vector.scalar_tensor_tensor(
                    yt[:], pff_ap(rt_), 2.0**-10, ob1v[:, rt_, :], Alu.mult, Alu.add
                )
                nc.vector.tensor_tensor(yt[:], yt[:], b2bc[:], Alu.add)
                layer_norm(
                    yt[:], yt[:], f"ln2_{rt_}",
                    gain_bc=g2bc if apply_g2b2 else None,
                    bias_bc=be2bc if apply_g2b2 else None,
                )
                nc.sync.dma_start(y_d[:, rt_ * D : (rt_ + 1) * D], yt[:])
        else:
            yts, s1s, s2s, mus, ucols = [], [], [], [], []
            for rt_ in range(RT):
                yt = ypool.tile([128, D], F32, name=f"y{rt_}", tag="y")
                s1 = spool.tile([128, 1], F32, name=f"s1_{rt_}", tag="stat")
                nc.vector.scalar_tensor_tensor(
                    yt[:], pff_ap(rt_), 2.0**-10, ob1v[:, rt_, :], Alu.mult, Alu.add,
                    accum_out=s1[:],
                )
                yts.append(yt); s1s.append(s1)
            for rt_ in range(RT):
                s2 = spool.tile([128, 1], F32, name=f"s2_{rt_}", tag="stat")
                nc.scalar.activation(
                    scr[rt_ % 2][:], yts[rt_][:],
                    mybir.ActivationFunctionType.Square, accum_out=s2[:],
                )
                s2s.append(s2)
            for rt_ in range(RT):
                # ucol = var = (s2 - s1^2/D)/D; mu = s1/D
                u = spool.tile([128, 1], F32, name=f"u{rt_}", tag="stat")
                nc.vector.tensor_tensor(u[:], s1s[rt_][:], s1s[rt_][:], Alu.mult)
                nc.vector.tensor_scalar(u[:], u[:], 1.0 / D, None, Alu.mult)
                nc.vector.tensor_tensor(u[:], s2s[rt_][:], u[:], Alu.subtract)
                nc.vector.tensor_scalar(u[:], u[:], 1.0 / D, None, Alu.mult)
                ucols.append(u)
                mu = spool.tile([128, 1], F32, name=f"mu{rt_}", tag="stat")
                nc.vector.tensor_scalar(mu[:], s1s[rt_][:], 1.0 / D, None, Alu.mult)
                mus.append(mu)
            rstdy = rsqrt_cols([u[:] for u in ucols], "l2")
            for rt_ in range(RT):
                nc.vector.tensor_scalar(
                    yts[rt_][:], yts[rt_][:], mus[rt_][:, 0:1],
                    rstdy[:, rt_ : rt_ + 1], Alu.subtract, Alu.mult,
                )
                if apply_g2b2:
                    nc.vector.tensor_tensor(yts[rt_][:], yts[rt_][:], g2bc[:], Alu.mult)
                    nc.vector.tensor_tensor(yts[rt_][:], yts[rt_][:], be2bc[:], Alu.add)
                nc.sync.dma_start(y_d[:, rt_ * D : (rt_ + 1) * D], yts[rt_][:])

    nc.compile()
    return nc


_CACHED = {}


def _get_nc(apply_g2b2: bool = False, add_b2: bool = False, b1_zero: bool = True):
    key = (apply_g2b2, add_b2, b1_zero)
    if key not in _CACHED:
        _CACHED[key] = build_program(*key)
    return _CACHED[key]
def _f8(x, scale_pow):
    return (np.asarray(x, np.float32) * (2.0**scale_pow)).astype(F8NP)


def _ktile_rows(a):
    """[K, M] -> [128, (K//128)*M]: out[p, j*M + m] = a[j*128 + p, m]."""
    K, M = a.shape
    return np.ascontiguousarray(
        a.reshape(K // 128, 128, M).transpose(1, 0, 2).reshape(128, -1)
    )


def kernel(**inputs) -> np.ndarray:
    x = np.asarray(inputs["inputs"], dtype=np.float32)
    enc = np.asarray(inputs["encoder_x"], dtype=np.float32)
    assert x.shape == (B, LQ, D) and enc.shape == (B, LK, D)
    assert int(np.asarray(inputs["n_heads"])) == H

    Wq = np.asarray(inputs["Wq"], np.float32)
    Wk = np.asarray(inputs["Wk"], np.float32)
    Wv = np.asarray(inputs["Wv"], np.float32)
    g1 = np.asarray(inputs["ln1_g"], np.float64)
    be1 = np.asarray(inputs["ln1_b"], np.float64)
    w1_raw = np.asarray(inputs["W1"], np.float64)
    w1_eff = (g1[:, None] * w1_raw).astype(np.float32)
    b1_eff = (np.asarray(inputs["b1"], np.float64) + be1 @ w1_raw).astype(np.float32)
    W2 = np.asarray(inputs["W2"], np.float32)
    b2 = np.asarray(inputs["b2"], np.float32)
    g2 = np.asarray(inputs["ln2_g"], np.float32)
    be2 = np.asarray(inputs["ln2_b"], np.float32)

    apply_g2b2 = not (np.allclose(g2, 1.0) and np.allclose(be2, 0.0))
    add_b2 = not np.allclose(b2, 0.0)
    b1_zero = bool(np.allclose(b1_eff, 0.0))
    nc = _get_nc(apply_g2b2, add_b2, b1_zero)

    shared = {
        "wq8": _ktile_rows(_f8(Wq, 5)),
        "wk8": _ktile_rows(_f8(Wk, 5)),
        "wv8": _ktile_rows(_f8(Wv, 5)),
        "w1b": _ktile_rows(_f8(w1_eff, 7)),
        "w2b": _ktile_rows(_f8(W2, 5)),
        "b1c": np.ascontiguousarray(
            _ktile_rows((b1_eff * 32.0)[:, None]).astype(np.float32)
        ),
        "g2": np.ascontiguousarray(g2),
        "be2": np.ascontiguousarray(be2),
        "b2": np.ascontiguousarray(b2),
    }
    xf = x.reshape(B * LQ, D)
    in_maps = []
    for c in range(N_CORES):
        b = c // (N_CORES // B)
        xs = xf[c * ROWS : (c + 1) * ROWS]
        m = dict(shared)
        m["xt8"] = _ktile_rows(_f8(np.ascontiguousarray(xs.T), 4))
        m["xb"] = _ktile_rows(xs.astype(BF16NP))
        m["enct8"] = _ktile_rows(_f8(np.ascontiguousarray(enc[b].T), 4))
        in_maps.append(m)

    res = run_bass_kernel_spmd(nc, in_maps, core_ids=list(range(N_CORES)))
    out = np.empty((B * LQ, D), np.float32)
    for c in range(N_CORES):
        yc = res.results[c]["y"].reshape(128, RT, D).transpose(1, 0, 2).reshape(ROWS, D)
        out[c * ROWS : (c + 1) * ROWS] = yc
    return out.reshape(B, LQ, D)
